# revision 1
# baseline (speedup 1.0000x reference)
"""Trainium2 Bass kernel for nn_DA_conv: per-sample dynamic depthwise 3x3 conv
(+LeakyReLU) followed by a 1x1 pointwise conv, with the 3x3 kernels produced by
a small per-sample MLP.

Strategy (8 NeuronCores, pure batch data-parallel, 2 samples per core):
  - SBUF layout: partition p = (sample s = p//64, channel c = p%64); the whole
    2-sample feature map lives resident in SBUF with zero-padded borders so
    every conv tap is a plain strided access-pattern read.
  - The kernel-generating MLP runs on the TensorEngine (tiny matmuls).
  - Depthwise 3x3 conv = 9 PSUM-accumulating diagonal matmuls per output tile.
    Diagonal 32x32 weight blocks + 32x32 TensorE array tiling (16 independent
    sub-tiles addressed via tile_position) recover the concurrency a depthwise
    contraction otherwise wastes on the 128x128 array.
  - LeakyReLU is fused into the PSUM->SBUF evacuation on the Scalar engine.
  - 1x1 conv = dense 32x32-tiled matmuls (contraction over channels), bias add
    fused into the PSUM->SBUF evacuation on the Vector engine.
  - Matmuls run in float32r (full-rate fp32 path; fp32 proper is 4x slower).
  - Emission is software-pipelined over half-blocks (depthwise of half m, then
    1x1 of half m-1) so PSUM evacuations overlap the next depthwise group.
"""

import os
import sys

sys.path.insert(0, "/opt/trn_rl_repo")

from contextlib import ExitStack

import numpy as np

import concourse.bacc as bacc
import concourse.bass as bass
import concourse.mybir as mybir
import concourse.tile as tile

S = 2            # samples per core
C = 64           # channels
H = W = 128      # spatial
KK = 3           # conv kernel size
NCORES = 8
RS = 132         # padded row stride in elements (16B-aligned: 132*4 = 528)
RP = H + 2       # padded row count (top/bottom halo)
XFREE = RP * RS  # padded image elements per partition
BR = 8           # image rows per block
NBLK = H // BR   # 16 blocks
HPX = (BR // 2) * W  # 512 pixels per half-block = one PSUM bank

f32 = mybir.dt.float32
f32r = mybir.dt.float32r
bf16 = mybir.dt.bfloat16
i32 = mybir.dt.int32

# x dtype for the depthwise matmuls. "f32r" keeps full fp32 DMA traffic;
# "bf16" halves the input DMA at a small accuracy cost.
X_MODE = os.environ.get("DA_CONV_X_MODE", "bf16")

LRELU = mybir.ActivationFunctionType.Lrelu
LRELU_MODE = os.environ.get("DA_CONV_LRELU", "prelu")
TAPS = [(di, dj) for di in range(KK) for dj in range(KK)]  # t = di*3 + dj


def build_program(x_mode: str = X_MODE) -> bass.Bass:
    # NOTE: fp32r matmuls cannot use TensorE column tiling on this toolchain
    # (s3d3_mm_valid_dst_partition), so the tiled conv stages must be bf16.
    xdt = bf16

    nc = bacc.Bacc("TRN2", target_bir_lowering=False, debug=False)

    x_d = nc.dram_tensor("x", [S * C, H * W], xdt, kind="ExternalInput").ap()
    dt_d = nc.dram_tensor("dT", [C, S], f32, kind="ExternalInput").ap()
    wk1_d = nc.dram_tensor("wk1t", [C, C], f32, kind="ExternalInput").ap()
    # Wk2 transposed + tap-major + duplicated over samples:
    # wk2td[j, t*128 + s*64 + c] = Wk2[c*9 + t, j]
    wk2_d = nc.dram_tensor("wk2td", [C, KK * KK * 2 * C], f32, kind="ExternalInput").ap()
    wct2_d = nc.dram_tensor("wct2", [2 * C, C], bf16, kind="ExternalInput").ap()
    bc_d = nc.dram_tensor("bc2", [2 * C, 1], f32, kind="ExternalInput").ap()
    out_d = nc.dram_tensor("out", [S * C, H * W], f32, kind="ExternalOutput").ap()

    with tile.TileContext(nc) as tc, ExitStack() as ctx:
        _body(ctx, tc, x_d, dt_d, wk1_d, wk2_d, wct2_d, bc_d, out_d, xdt)
    nc.compile()
    return nc


def _body(ctx, tc, x_d, dt_d, wk1_d, wk2_d, wct2_d, bc_d, out_d, xdt):
    nc = tc.nc
    const = ctx.enter_context(tc.tile_pool(name="const", bufs=1))
    xpool = ctx.enter_context(tc.tile_pool(name="xs", bufs=1))
    dwlp = ctx.enter_context(tc.tile_pool(name="dwl", bufs=4))
    abtp = ctx.enter_context(tc.tile_pool(name="abt", bufs=4))
    o2p = ctx.enter_context(tc.tile_pool(name="o2", bufs=NBLK // 2))
    pdw = ctx.enter_context(tc.tile_pool(name="pdw", bufs=2, space="PSUM"))
    po2 = ctx.enter_context(tc.tile_pool(name="po2", bufs=2, space="PSUM"))

    # ---------------- small-weight loads ----------------
    wk1t = const.tile([C, C], f32)
    nc.sync.dma_start(wk1t[:, :], wk1_d)
    wk2td = const.tile([C, KK * KK * 2 * C], f32)
    nc.sync.dma_start(wk2td[:, :], wk2_d)
    dts = const.tile([C, S], f32)
    nc.sync.dma_start(dts[:, :], dt_d)
    wct2 = const.tile([2 * C, C], bf16)
    nc.sync.dma_start(wct2[:, :], wct2_d)
    bc2 = const.tile([2 * C, 1], f32)
    nc.sync.dma_start(bc2[:, :], bc_d)

    # ---------------- kernel-generating MLP ----------------
    # hid[j, s] = lrelu(sum_i Wk1[j, i] d[s, i])  via lhsT = Wk1.T
    hid_ps = po2.tile([C, S], f32, tag="oe")
    nc.tensor.matmul(
        hid_ps[:, :], lhsT=wk1t[:, :], rhs=dts[:, :], start=True, stop=True,
    )
    hid_sb = const.tile([C, S], f32)
    if LRELU_MODE == "prelu":
        nc.scalar.activation(hid_sb[:, :], hid_ps[:, :],
                             mybir.ActivationFunctionType.Prelu, alpha=0.1)
    else:
        hid_ab = const.tile([C, S], f32)
        nc.scalar.activation(hid_ab[:, :], hid_ps[:, :],
                             mybir.ActivationFunctionType.Abs, scale=0.45)
        nc.vector.scalar_tensor_tensor(
            hid_sb[:, :], hid_ps[:, :], 0.55, hid_ab[:, :],
            op0=mybir.AluOpType.mult, op1=mybir.AluOpType.add,
        )

    # kern tap columns: kcols[s*64+c, t] = kern[s, c*9+t]
    kcols = const.tile([2 * C, KK * KK], f32)
    for t in range(KK * KK):
        kp = po2.tile([2 * C, S], f32, tag="oe")
        nc.tensor.matmul(
            kp[:, :],
            lhsT=wk2td[:, t * 128 : (t + 1) * 128],
            rhs=hid_sb[:, :],
            start=True, stop=True,
        )
        # partition p wants free column s = p//64 of kp (partition-aligned copies)
        nc.vector.tensor_copy(kcols[0:C, t : t + 1], kp[0:C, 0:1])
        nc.vector.tensor_copy(kcols[C : 2 * C, t : t + 1], kp[C : 2 * C, 1:2])

    # identity -> per-tap diagonal weight matrices diag[:, t*128:(t+1)*128]
    id_i = const.tile([128, 128], i32)
    nc.gpsimd.iota(id_i[:, :], pattern=[[1, 128]], base=0, channel_multiplier=-1)
    idf = const.tile([128, 128], f32)
    nc.vector.tensor_scalar(idf[:, :], id_i[:, :], 0, None, mybir.AluOpType.is_equal)
    diag = const.tile([128, KK * KK * 128], xdt)
    for t in range(KK * KK):
        nc.vector.tensor_scalar_mul(
            diag[:, t * 128 : (t + 1) * 128], idf[:, :], kcols[:, t : t + 1]
        )

    # ---------------- resident padded feature map ----------------
    xs = xpool.tile([128, XFREE], xdt)
    # top halo row + row-1 left pad (contiguous), bottom halo row, and the
    # pad columns: right-pad of row r is contiguous with left-pad of row r+1,
    # so one strided memset covers all interior pad columns.
    nc.vector.memset(xs[:, 0 : RS + 1], 0.0)
    nc.vector.memset(xs[:, (RP - 1) * RS : RP * RS], 0.0)
    pads = xs[:, W + 1 : W + 1 + (H + 1) * RS].rearrange("p (r w) -> p r w", w=RS)
    nc.vector.memset(pads[:, :, 0:4], 0.0)
    # image rows in 16 chunks so compute can start early
    for k in range(NBLK):
        src = x_d[:, k * BR * W : (k + 1) * BR * W].rearrange(
            "p (r w) -> p r w", w=W
        )
        o = (k * BR + 1) * RS + 1
        dst = xs[:, o : o + BR * RS].rearrange("p (r w) -> p r w", w=RS)[:, :, 0:W]
        nc.sync.dma_start(dst, src)

    # ---------------- main loop ----------------
    # 64x64 TensorE tiling: 4 concurrent positions. Each PSUM bank has exactly
    # one row-tile writer (HW constraint): P_A <- row tile 0 (sample A
    # channels), P_B <- row tile 1; column groups select the pixel half (E =
    # rows 8k..8k+3, O = rows 8k+4..8k+7) within the bank.
    xrows = xs[:, :].rearrange("p (r w) -> p r w", w=RS)

    def lrelu_evac(D, P):
        if LRELU_MODE == "prelu":
            nc.scalar.activation(D[:, :], P[:, :],
                                 mybir.ActivationFunctionType.Prelu, alpha=0.1)
        else:
            # lrelu(x) = 0.55x + 0.45|x| ; Abs on ScalarE, fused MAC on VectorE
            ab = abtp.tile([128, HPX], f32, tag="abt")
            nc.scalar.activation(ab[:, :], P[:, :],
                                 mybir.ActivationFunctionType.Abs, scale=0.45)
            nc.vector.scalar_tensor_tensor(
                D[:, :], P[:, :], 0.55, ab[:, :],
                op0=mybir.AluOpType.mult, op1=mybir.AluOpType.add,
            )

    def dw_stage(k):
        r0e = BR * k
        r0o = BR * k + BR // 2
        PA = pdw.tile([128, HPX], f32, tag="pa")
        PB = pdw.tile([128, HPX], f32, tag="pb")
        for t, (di, dj) in enumerate(TAPS):
            wE = xrows[:, r0e + di : r0e + di + 4, dj : dj + W]
            wO = xrows[:, r0o + di : r0o + di + 4, dj : dj + W]
            la = diag[0:C, t * 128 : t * 128 + C]
            lb = diag[C : 2 * C, t * 128 + C : t * 128 + 2 * C]
            for cg, win in ((0, wE), (C, wO)):
                nc.tensor.matmul(
                    PA[cg : cg + C, :], lhsT=la, rhs=win[0:C, :, :],
                    start=(t == 0), stop=(t == KK * KK - 1),
                    tile_position=(0, cg), skip_group_check=True,
                )
                nc.tensor.matmul(
                    PB[cg : cg + C, :], lhsT=lb, rhs=win[C : 2 * C, :, :],
                    start=(t == 0), stop=(t == KK * KK - 1),
                    tile_position=(C, cg), skip_group_check=True,
                )
        DA = dwlp.tile([128, HPX], bf16, tag="da")
        DB = dwlp.tile([128, HPX], bf16, tag="db")
        lrelu_evac(DA, PA)
        lrelu_evac(DB, PB)
        return k, DA, DB

    def conv1x1_stage(k, DA, DB):
        OE = po2.tile([128, HPX], f32, tag="oe")
        OO = po2.tile([128, HPX], f32, tag="oo")
        # E outputs via row tile 0, O outputs via row tile 1; standard [A;B]
        # channel layout lands directly in each output bank.
        nc.tensor.matmul(OE[0:C, :], lhsT=wct2[0:C, :], rhs=DA[0:C, :],
                         start=True, stop=True, tile_position=(0, 0),
                         skip_group_check=True)
        nc.tensor.matmul(OE[C : 2 * C, :], lhsT=wct2[0:C, :], rhs=DB[0:C, :],
                         start=True, stop=True, tile_position=(0, C),
                         skip_group_check=True)
        nc.tensor.matmul(OO[0:C, :], lhsT=wct2[C : 2 * C, :], rhs=DA[C : 2 * C, :],
                         start=True, stop=True, tile_position=(C, 0),
                         skip_group_check=True)
        nc.tensor.matmul(OO[C : 2 * C, :], lhsT=wct2[C : 2 * C, :],
                         rhs=DB[C : 2 * C, :],
                         start=True, stop=True, tile_position=(C, C),
                         skip_group_check=True)
        # bias add into the (128, 2048) staging tile; 1 MiB output DMA / 2 blocks
        q, qi = divmod(k, 2)
        if qi == 0:
            zcur["t"] = o2p.tile([128, 4 * HPX], f32, tag="o2", name=f"zt{k}")
        zt = zcur["t"]
        zb = 2 * qi * HPX
        nc.vector.tensor_scalar_add(zt[:, zb : zb + HPX], OE[:, :], bc2[:, 0:1])
        nc.vector.tensor_scalar_add(
            zt[:, zb + HPX : zb + 2 * HPX], OO[:, :], bc2[:, 0:1]
        )
        if qi == 1:
            nc.sync.dma_start(out_d[:, q * 4 * HPX : (q + 1) * 4 * HPX], zt[:, :])

    pending = None
    zcur = {"t": None}
    for k in range(NBLK):
        st = dw_stage(k)
        if pending is not None:
            conv1x1_stage(*pending)
        pending = st
    conv1x1_stage(*pending)


# ---------------------------------------------------------------------------
# host-side entry point
# ---------------------------------------------------------------------------

_PROGRAM_CACHE: dict[str, bass.Bass] = {}


def _get_program(x_mode: str) -> bass.Bass:
    if x_mode not in _PROGRAM_CACHE:
        _PROGRAM_CACHE[x_mode] = build_program(x_mode)
    return _PROGRAM_CACHE[x_mode]


def _host_prep(inputs: dict, x_mode: str):
    import ml_dtypes

    x = np.asarray(inputs["x"], dtype=np.float32)
    d = np.asarray(inputs["d"], dtype=np.float32)
    Wk1 = np.asarray(inputs["Wk1"], dtype=np.float32)
    Wk2 = np.asarray(inputs["Wk2"], dtype=np.float32)
    Wc = np.asarray(inputs["Wc"], dtype=np.float32)
    bc = np.asarray(inputs["bc"], dtype=np.float32)

    wk1t = np.ascontiguousarray(Wk1.T)
    w = Wk2.reshape(C, KK * KK, C).transpose(2, 1, 0)  # (j, t, c)
    wk2td = np.ascontiguousarray(
        np.concatenate([w, w], axis=2).reshape(C, KK * KK * 2 * C)
    )
    wct = np.ascontiguousarray(Wc.T)
    wct2 = np.ascontiguousarray(np.concatenate([wct, wct], axis=0)).astype(
        ml_dtypes.bfloat16
    )
    bc2 = np.ascontiguousarray(np.concatenate([bc, bc]).reshape(2 * C, 1))

    xcast = x.astype(ml_dtypes.bfloat16)

    in_maps = []
    for i in range(NCORES):
        xs = np.ascontiguousarray(xcast[S * i : S * (i + 1)].reshape(S * C, H * W))
        dT = np.ascontiguousarray(d[S * i : S * (i + 1)].T)
        in_maps.append(
            {
                "x": xs,
                "dT": dT,
                "wk1t": wk1t,
                "wk2td": wk2td,
                "wct2": wct2,
                "bc2": bc2,
            }
        )
    return in_maps


def run_on_hw(inputs: dict, x_mode: str = None, **kwargs):
    """Run the SPMD kernel on 8 NeuronCores; returns (output, BassKernelResults)."""
    from concourse.bass_utils import run_bass_kernel_spmd

    x_mode = x_mode or X_MODE
    nc = _get_program(x_mode)
    in_maps = _host_prep(inputs, x_mode)
    res = run_bass_kernel_spmd(nc, in_maps, core_ids=list(range(NCORES)), **kwargs)
    outs = res.results
    B = S * NCORES
    out = np.empty((B, C, H, W), dtype=np.float32)
    for i in range(NCORES):
        out[S * i : S * (i + 1)] = outs[i]["out"].reshape(S, C, H, W)
    return out, res


def kernel(**inputs) -> np.ndarray:
    out, _ = run_on_hw(inputs)
    return out


if __name__ == "__main__":
    nc = build_program()
    print("program built OK")



# revision 5
# speedup vs baseline: 2.0834x; 2.0834x over previous
"""Trainium2 Bass kernel for nn_DA_conv: per-sample dynamic depthwise 3x3 conv
(+LeakyReLU) followed by a 1x1 pointwise conv, with the 3x3 kernels produced by
a small per-sample MLP.

Strategy (8 NeuronCores, pure batch data-parallel, 2 samples per core):
  - SBUF partition p = (sample s = p//64, channel c = p%64); the 2-sample
    feature map lives resident in SBUF with zero-padded borders.
  - The depthwise conv work is split across engines by image-row region:
      * DP rows (top):   per-tap products on DVE (tensor_scalar_mul, 4x bf16
        mode), add tree on Pool (tensor_tensor), LeakyReLU on Act.
      * D rows (middle): products + add tree on DVE, LeakyReLU on Act.
      * PE rows (rest):  9 PSUM-accumulating full-128-partition diagonal
        matmuls per 512-px tile, Prelu evacuation on Act.
  - 1x1 conv = block-diagonal [128x128] bf16 matmuls (both samples per
    instruction); PSUM evacuated by Act with the bias add fused (Identity
    activation + per-partition bias), written as bf16.
  - Input x and output travel as bf16 (fp32 restored on host).
  - x is loaded over two DMA queues (SP: PE-region rows; Act: DP/D rows) so
    every engine's first chunk lands early.
"""

import sys

sys.path.insert(0, "/opt/trn_rl_repo")

from contextlib import ExitStack

import numpy as np

import concourse.bacc as bacc
import concourse.bass as bass
import concourse.mybir as mybir
import concourse.tile as tile

S = 2            # samples per core
C = 64           # channels
H = W = 128      # spatial
KK = 3           # conv kernel size
NCORES = 8
RS = 132         # padded row stride in elements (16B-aligned: 132*4 = 528)
RP = H + 2       # padded row count (top/bottom halo)
XFREE = RP * RS  # padded image elements per partition

f32 = mybir.dt.float32
bf16 = mybir.dt.bfloat16
i32 = mybir.dt.int32

LRELU = mybir.ActivationFunctionType.Prelu
TAPS = [(di, dj) for di in range(KK) for dj in range(KK)]  # t = di*3 + dj

# ---- region assignment (rows of the 128-row image) ----
# rows [0, DP_END): DVE-mul + Pool-add;  [DP_END, D_END): DVE only;
# [D_END, 128): TensorEngine diagonal matmuls.
DP_CHUNKS = [(0, 16), (16, 16)]
D_CHUNKS = [(32, 16), (48, 12)]
PE_GROUPS = [60 + 4 * g for g in range(17)]
ADD_TREE = [  # (dst, src) pairs over 9 product slots; acc ends in slot 0
    (0, 1), (2, 3), (4, 5), (6, 7), (0, 2), (4, 6), (0, 4), (0, 8),
]


def build_program() -> bass.Bass:
    nc = bacc.Bacc("TRN2", target_bir_lowering=False, debug=False)

    x_d = nc.dram_tensor("x", [S * C, H * W], bf16, kind="ExternalInput").ap()
    dt_d = nc.dram_tensor("dT", [C, S], f32, kind="ExternalInput").ap()
    wk1_d = nc.dram_tensor("wk1t", [C, C], f32, kind="ExternalInput").ap()
    # Wk2 transposed + tap-major + duplicated over samples:
    # wk2td[j, t*128 + s*64 + c] = Wk2[c*9 + t, j]
    wk2_d = nc.dram_tensor("wk2td", [C, KK * KK * 2 * C], f32, kind="ExternalInput").ap()
    # block-diagonal 1x1 weights: wcb[(s,ci),(s,co)] = Wc[co,ci]
    wcb_d = nc.dram_tensor("wcb", [2 * C, 2 * C], bf16, kind="ExternalInput").ap()
    bc_d = nc.dram_tensor("bc2", [2 * C, 1], f32, kind="ExternalInput").ap()
    out_d = nc.dram_tensor("out", [S * C, H * W], bf16, kind="ExternalOutput").ap()

    with tile.TileContext(nc) as tc, ExitStack() as ctx:
        _body(ctx, tc, x_d, dt_d, wk1_d, wk2_d, wcb_d, bc_d, out_d)
    nc.compile()
    return nc


def _body(ctx, tc, x_d, dt_d, wk1_d, wk2_d, wcb_d, bc_d, out_d):
    nc = tc.nc
    const = ctx.enter_context(tc.tile_pool(name="const", bufs=1))
    xpool = ctx.enter_context(tc.tile_pool(name="xs", bufs=1))
    dgp = ctx.enter_context(tc.tile_pool(name="dg", bufs=1))
    prodp = ctx.enter_context(tc.tile_pool(name="prod", bufs=2))
    accp = ctx.enter_context(tc.tile_pool(name="acc", bufs=2))
    ostg = ctx.enter_context(tc.tile_pool(name="ostg", bufs=4))
    pdw = ctx.enter_context(tc.tile_pool(name="pdw", bufs=3, space="PSUM"))
    po2 = ctx.enter_context(tc.tile_pool(name="po2", bufs=2, space="PSUM"))

    # ---------------- small-weight loads (SP queue, first) ----------------
    wk1t = const.tile([C, C], f32)
    nc.sync.dma_start(wk1t[:, :], wk1_d)
    wk2td = const.tile([C, KK * KK * 2 * C], f32)
    nc.sync.dma_start(wk2td[:, :], wk2_d)
    dts = const.tile([C, S], f32)
    nc.sync.dma_start(dts[:, :], dt_d)
    wcb = const.tile([2 * C, 2 * C], bf16)
    nc.sync.dma_start(wcb[:, :], wcb_d)
    bc2 = const.tile([2 * C, 1], f32)
    nc.sync.dma_start(bc2[:, :], bc_d)

    # ---------------- resident padded feature map ----------------
    xs = xpool.tile([128, XFREE], bf16)
    # top halo row + row-1 left pad, bottom halo row, and all interior pad
    # columns (right pad of row r is contiguous with left pad of row r+1).
    nc.vector.memset(xs[:, 0 : RS + 1], 0.0)
    nc.vector.memset(xs[:, (RP - 1) * RS : RP * RS], 0.0)
    pads = xs[:, W + 1 : W + 1 + (H + 1) * RS].rearrange("p (r w) -> p r w", w=RS)
    nc.vector.memset(pads[:, :, 0:4], 0.0)

    def load_x(engine, r0, nr):
        src = x_d[:, r0 * W : (r0 + nr) * W].rearrange("p (r w) -> p r w", w=W)
        o = (r0 + 1) * RS + 1
        dst = xs[:, o : o + nr * RS].rearrange("p (r w) -> p r w", w=RS)[:, :, 0:W]
        engine.dma_start(dst, src)

    # PE-region rows on the SP queue (PE consumes them first)
    load_x(nc.sync, 56, 24)
    load_x(nc.sync, 80, 24)
    load_x(nc.sync, 104, 24)
    # DP/D-region rows on the Act queue
    load_x(nc.scalar, 0, 20)

    # ---------------- kernel-generating MLP ----------------
    hid_ps = po2.tile([C, S], f32, tag="oo")
    nc.tensor.matmul(
        hid_ps[:, :], lhsT=wk1t[:, :], rhs=dts[:, :], start=True, stop=True,
    )
    hid_sb = const.tile([C, S], f32)
    nc.scalar.activation(hid_sb[:, :], hid_ps[:, :], LRELU, alpha=0.1)

    load_x(nc.scalar, 20, 20)
    load_x(nc.scalar, 40, 16)

    # kern tap columns: kcols[s*64+c, t] = kern[s, c*9+t]
    kcols = const.tile([2 * C, KK * KK], f32)
    for t in range(KK * KK):
        kp = po2.tile([2 * C, S], f32, tag="oo")
        nc.tensor.matmul(
            kp[:, :],
            lhsT=wk2td[:, t * 128 : (t + 1) * 128],
            rhs=hid_sb[:, :],
            start=True, stop=True,
        )
        nc.vector.tensor_copy(kcols[0:C, t : t + 1], kp[0:C, 0:1])
        nc.vector.tensor_copy(kcols[C : 2 * C, t : t + 1], kp[C : 2 * C, 1:2])

    # identity -> per-tap diagonal weight matrices diag[:, t*128:(t+1)*128]
    id_i = const.tile([128, 128], i32)
    nc.gpsimd.iota(id_i[:, :], pattern=[[1, 128]], base=0, channel_multiplier=-1)
    idf = const.tile([128, 128], f32)
    nc.vector.tensor_scalar(idf[:, :], id_i[:, :], 0, None, mybir.AluOpType.is_equal)
    diag = const.tile([128, KK * KK * 128], bf16)
    for t in range(KK * KK):
        nc.vector.tensor_scalar_mul(
            diag[:, t * 128 : (t + 1) * 128], idf[:, :], kcols[:, t : t + 1]
        )

    # ---------------- main loop ----------------
    xrows = xs[:, :].rearrange("p (r w) -> p r w", w=RS)

    def win(r0, nr, di, dj):
        # image rows r0..r0+nr-1 under tap (di,dj); padded row r0+di covers
        # image row r0+di-1 (the +1 pad offset cancels the tap's -1).
        return xrows[:, r0 + di : r0 + di + nr, dj : dj + W]

    dg = {}  # image row -> (tile, px offset) for 4-row (512 px) slices

    def set_dg(r0, nr, tilev):
        for i in range(nr // 4):
            dg[r0 + 4 * i] = (tilev, 512 * i)

    def pe_group(r0):
        P = pdw.tile([128, 512], f32, tag="pdw", name=f"pdw{r0}")
        for t, (di, dj) in enumerate(TAPS):
            nc.tensor.matmul(
                P[:, :],
                lhsT=diag[:, t * 128 : (t + 1) * 128],
                rhs=win(r0, 4, di, dj),
                start=(t == 0), stop=(t == KK * KK - 1),
            )
        D = dgp.tile([128, 512], bf16, name=f"dpe{r0}")
        nc.scalar.activation(D[:, :], P[:, :], LRELU, alpha=0.1)
        set_dg(r0, 4, D)

    def dve_muls(r0, nr):
        px = nr * W
        prod = prodp.tile([128, 9 * px], bf16, tag="prod", name=f"prod{r0}")
        p3 = prod[:, :].rearrange("p (t x) -> p t x", x=px)
        for t, (di, dj) in enumerate(TAPS):
            o = p3[:, t, :].rearrange("p (r w) -> p r w", w=W)
            nc.vector.tensor_scalar_mul(o, win(r0, nr, di, dj), kcols[:, t : t + 1])
        return p3

    def adds_and_lrelu(eng, p3, r0, nr):
        px = nr * W
        for dst, src in ADD_TREE[:-1]:
            eng.tensor_tensor(
                p3[:, dst, :], p3[:, dst, :], p3[:, src, :], op=mybir.AluOpType.add
            )
        acc = accp.tile([128, px], bf16, tag="acc", name=f"acc{r0}")
        eng.tensor_tensor(
            acc[:, :], p3[:, 0, :], p3[:, 8, :], op=mybir.AluOpType.add
        )
        D = dgp.tile([128, px], bf16, name=f"dd{r0}")
        nc.scalar.activation(D[:, :], acc[:, :], LRELU, alpha=0.1)
        set_dg(r0, nr, D)

    # --- 1x1 span (8 rows = 1024 px) + bias evac; out DMA every 2 spans ---
    ost = {"t": None}

    def span_1x1(s):
        r0 = 8 * s
        O = po2.tile([128, 1024], f32, tag="oo", name=f"o2{s}")
        for h in range(2):
            t_, off = dg[r0 + 4 * h]
            nc.tensor.matmul(
                O[:, 512 * h : 512 * (h + 1)],
                lhsT=wcb[:, :], rhs=t_[:, off : off + 512],
                start=True, stop=True,
            )
        if s % 2 == 0:
            ost["t"] = ostg.tile([128, 2048], bf16, tag="ostg", name=f"ostg{s}")
        z = ost["t"]
        nc.scalar.add(z[:, 1024 * (s % 2) : 1024 * (s % 2 + 1)], O[:, :], bc2[:, 0:1])
        if s % 2 == 1:
            q = s // 2
            nc.gpsimd.dma_start(out_d[:, q * 2048 : (q + 1) * 2048], z[:, :])

    # ---------------- schedule ----------------
    # DVE stream: DP muls first (pool is the long pole), then D chunks.
    dp_p3 = []
    for r0, nr in DP_CHUNKS:
        dp_p3.append(dve_muls(r0, nr))
    # Pool stream: add trees for DP chunks.
    for (r0, nr), p3 in zip(DP_CHUNKS, dp_p3):
        adds_and_lrelu(nc.gpsimd, p3, r0, nr)
    # D chunks fully on DVE.
    for r0, nr in D_CHUNKS:
        p3 = dve_muls(r0, nr)
        adds_and_lrelu(nc.vector, p3, r0, nr)

    # PE stream: dw groups; interleave 1x1 spans as their inputs appear.
    done_rows = set()
    emitted_spans = set()

    def flush_spans():
        for s in range(16):
            if s in emitted_spans:
                continue
            if (8 * s in done_rows) and (8 * s + 4 in done_rows):
                span_1x1(s)
                emitted_spans.add(s)

    for r0, nr in DP_CHUNKS + D_CHUNKS:
        for i in range(nr // 4):
            done_rows.add(r0 + 4 * i)
    for r0 in PE_GROUPS:
        pe_group(r0)
        done_rows.add(r0)
        flush_spans()
    flush_spans()
    assert len(emitted_spans) == 16, f"unscheduled spans: {sorted(emitted_spans)}"


# ---------------------------------------------------------------------------
# host-side entry point
# ---------------------------------------------------------------------------

_PROGRAM_CACHE: dict[str, bass.Bass] = {}


def _get_program() -> bass.Bass:
    if "p" not in _PROGRAM_CACHE:
        _PROGRAM_CACHE["p"] = build_program()
    return _PROGRAM_CACHE["p"]


def _host_prep(inputs: dict):
    import ml_dtypes

    x = np.asarray(inputs["x"], dtype=np.float32)
    d = np.asarray(inputs["d"], dtype=np.float32)
    Wk1 = np.asarray(inputs["Wk1"], dtype=np.float32)
    Wk2 = np.asarray(inputs["Wk2"], dtype=np.float32)
    Wc = np.asarray(inputs["Wc"], dtype=np.float32)
    bc = np.asarray(inputs["bc"], dtype=np.float32)

    wk1t = np.ascontiguousarray(Wk1.T)
    w = Wk2.reshape(C, KK * KK, C).transpose(2, 1, 0)  # (j, t, c)
    wk2td = np.ascontiguousarray(
        np.concatenate([w, w], axis=2).reshape(C, KK * KK * 2 * C)
    )
    wcb = np.zeros((2 * C, 2 * C), dtype=np.float32)
    wcb[0:C, 0:C] = Wc.T
    wcb[C:, C:] = Wc.T
    wcb = wcb.astype(ml_dtypes.bfloat16)
    bc2 = np.ascontiguousarray(np.concatenate([bc, bc]).reshape(2 * C, 1))

    xcast = x.astype(ml_dtypes.bfloat16)

    in_maps = []
    for i in range(NCORES):
        xs = np.ascontiguousarray(xcast[S * i : S * (i + 1)].reshape(S * C, H * W))
        dT = np.ascontiguousarray(d[S * i : S * (i + 1)].T)
        in_maps.append(
            {
                "x": xs,
                "dT": dT,
                "wk1t": wk1t,
                "wk2td": wk2td,
                "wcb": wcb,
                "bc2": bc2,
            }
        )
    return in_maps


def run_on_hw(inputs: dict, **kwargs):
    """Run the SPMD kernel on 8 NeuronCores; returns (output, results)."""
    from concourse.bass_utils import run_bass_kernel_spmd

    nc = _get_program()
    in_maps = _host_prep(inputs)
    res = run_bass_kernel_spmd(nc, in_maps, core_ids=list(range(NCORES)), **kwargs)
    outs = res.results
    B = S * NCORES
    out = np.empty((B, C, H, W), dtype=np.float32)
    for i in range(NCORES):
        out[S * i : S * (i + 1)] = outs[i]["out"].astype(np.float32).reshape(
            S, C, H, W
        )
    return out, res


def kernel(**inputs) -> np.ndarray:
    out, _ = run_on_hw(inputs)
    return out


if __name__ == "__main__":
    nc = build_program()
    print("program built OK")


# revision 7
# speedup vs baseline: 2.2350x; 1.0728x over previous
"""Trainium2 Bass kernel for nn_DA_conv: per-sample dynamic depthwise 3x3 conv
(+LeakyReLU) followed by a 1x1 pointwise conv, with the 3x3 kernels produced by
a small per-sample MLP.

Strategy (8 NeuronCores, pure batch data-parallel, 2 samples per core):
  - SBUF partition p = (sample s = p//64, channel c = p%64); the 2-sample
    feature map lives resident in SBUF, zero-padded ON THE HOST so the DMA in
    is fully contiguous (1 descriptor per partition per chunk).
  - Depthwise conv split across engines by image-row region:
      * PE rows:  9 PSUM-accumulating full-128-partition diagonal matmuls per
        512-px tile; Prelu evacuation (1024 px) on Act.
      * DP rows:  per-tap products on DVE (tensor_scalar_mul, 4x bf16 mode),
        add tree on Pool (tensor_tensor), LeakyReLU on Act.
      * D rows:   products + add tree fully on DVE, LeakyReLU on Act.
  - 1x1 conv = block-diagonal [128x128] bf16 matmuls; PSUM evacuated by Act
    with the bias add fused (Identity + per-partition bias), written bf16.
  - All DMA transfers are engine-time in this machine model, so x chunks are
    split between the SP and Act queues in consumption order; output DMAs
    ride SP.
"""

import sys

sys.path.insert(0, "/opt/trn_rl_repo")

from contextlib import ExitStack

import numpy as np

import concourse.bacc as bacc
import concourse.bass as bass
import concourse.mybir as mybir
import concourse.tile as tile

S = 2            # samples per core
C = 64           # channels
H = W = 128      # spatial
KK = 3           # conv kernel size
NCORES = 8
RS = 132         # padded row stride in elements
RP = H + 2       # padded row count (top/bottom halo)
XFREE = RP * RS  # padded image elements per partition

f32 = mybir.dt.float32
bf16 = mybir.dt.bfloat16
i32 = mybir.dt.int32

LRELU = mybir.ActivationFunctionType.Prelu
TAPS = [(di, dj) for di in range(KK) for dj in range(KK)]  # t = di*3 + dj

# ---- region assignment (rows of the 128-row image) ----
PE_GROUPS = [4 * g for g in range(17)]              # rows 0..67
DP_CHUNKS = [(68, 4), (72, 12), (84, 12), (96, 8)]  # DVE muls + Pool adds
D_CHUNKS = [(104, 12), (116, 12)]                   # all-DVE
ADD_TREE = [  # (dst, src) pairs over 9 product slots; acc ends in slot 0
    (0, 1), (2, 3), (4, 5), (6, 7), (0, 2), (4, 6), (0, 4), (0, 8),
]
# x chunks in PADDED row space: (first padded row, count, queue)
# padded row pr holds image row pr-1.
X_CHUNKS_SP = [(0, 18), (46, 28), (18, 28), (106, 24)]
X_CHUNKS_ACT = [(74, 16), (90, 16)]


def build_program() -> bass.Bass:
    nc = bacc.Bacc("TRN2", target_bir_lowering=False, debug=False)

    x_d = nc.dram_tensor("xpad", [S * C, XFREE], bf16, kind="ExternalInput").ap()
    dt_d = nc.dram_tensor("dT", [C, S], f32, kind="ExternalInput").ap()
    wk1_d = nc.dram_tensor("wk1t", [C, C], f32, kind="ExternalInput").ap()
    # Wk2 transposed + tap-major + duplicated over samples:
    # wk2td[j, t*128 + s*64 + c] = Wk2[c*9 + t, j]
    wk2_d = nc.dram_tensor("wk2td", [C, KK * KK * 2 * C], f32, kind="ExternalInput").ap()
    # block-diagonal 1x1 weights: wcb[(s,ci),(s,co)] = Wc[co,ci]
    wcb_d = nc.dram_tensor("wcb", [2 * C, 2 * C], bf16, kind="ExternalInput").ap()
    bc_d = nc.dram_tensor("bc2", [2 * C, 1], f32, kind="ExternalInput").ap()
    out_d = nc.dram_tensor("out", [S * C, H * W], bf16, kind="ExternalOutput").ap()

    with tile.TileContext(nc) as tc, ExitStack() as ctx:
        _body(ctx, tc, x_d, dt_d, wk1_d, wk2_d, wcb_d, bc_d, out_d)
    nc.compile()
    return nc


def _body(ctx, tc, x_d, dt_d, wk1_d, wk2_d, wcb_d, bc_d, out_d):
    nc = tc.nc
    const = ctx.enter_context(tc.tile_pool(name="const", bufs=1))
    xpool = ctx.enter_context(tc.tile_pool(name="xs", bufs=1))
    dgp = ctx.enter_context(tc.tile_pool(name="dg", bufs=1))
    dpprod = ctx.enter_context(tc.tile_pool(name="dpprod", bufs=2))
    dprod = ctx.enter_context(tc.tile_pool(name="dprod", bufs=1))
    accp = ctx.enter_context(tc.tile_pool(name="acc", bufs=2))
    ostg = ctx.enter_context(tc.tile_pool(name="ostg", bufs=4))
    pdw = ctx.enter_context(tc.tile_pool(name="pdw", bufs=2, space="PSUM"))
    po2 = ctx.enter_context(tc.tile_pool(name="po2", bufs=2, space="PSUM"))

    # ---------------- input loads ----------------
    # MLP weights on the Act queue first (they gate kcols/diag), then the
    # mid-image x chunks; SP carries the rest of x, then wcb/bc2.
    dts = const.tile([C, S], f32)
    nc.scalar.dma_start(dts[:, :], dt_d)
    wk1t = const.tile([C, C], f32)
    nc.scalar.dma_start(wk1t[:, :], wk1_d)
    wk2td = const.tile([C, KK * KK * 2 * C], f32)
    nc.scalar.dma_start(wk2td[:, :], wk2_d)

    xs = xpool.tile([128, XFREE], bf16)

    def load_x(engine, pr0, npr):
        engine.dma_start(
            xs[:, pr0 * RS : (pr0 + npr) * RS], x_d[:, pr0 * RS : (pr0 + npr) * RS]
        )

    load_x(nc.sync, *X_CHUNKS_SP[0])
    load_x(nc.sync, *X_CHUNKS_SP[1])
    load_x(nc.scalar, *X_CHUNKS_ACT[0])
    load_x(nc.scalar, *X_CHUNKS_ACT[1])
    load_x(nc.sync, *X_CHUNKS_SP[2])
    load_x(nc.sync, *X_CHUNKS_SP[3])

    wcb = const.tile([2 * C, 2 * C], bf16)
    nc.sync.dma_start(wcb[:, :], wcb_d)
    bc2 = const.tile([2 * C, 1], f32)
    nc.sync.dma_start(bc2[:, :], bc_d)

    # ---------------- kernel-generating MLP ----------------
    hid_ps = po2.tile([C, S], f32, tag="oo")
    nc.tensor.matmul(
        hid_ps[:, :], lhsT=wk1t[:, :], rhs=dts[:, :], start=True, stop=True,
    )
    hid_sb = const.tile([C, S], f32)
    nc.scalar.activation(hid_sb[:, :], hid_ps[:, :], LRELU, alpha=0.1)

    # kern tap columns: kcols[s*64+c, t] = kern[s, c*9+t]
    kcols = const.tile([2 * C, KK * KK], f32)
    for t in range(KK * KK):
        kp = po2.tile([2 * C, S], f32, tag="oo")
        nc.tensor.matmul(
            kp[:, :],
            lhsT=wk2td[:, t * 128 : (t + 1) * 128],
            rhs=hid_sb[:, :],
            start=True, stop=True,
        )
        nc.vector.tensor_copy(kcols[0:C, t : t + 1], kp[0:C, 0:1])
        nc.vector.tensor_copy(kcols[C : 2 * C, t : t + 1], kp[C : 2 * C, 1:2])

    # identity -> per-tap diagonal weight matrices diag[:, t*128:(t+1)*128]
    id_i = const.tile([128, 128], i32)
    nc.gpsimd.iota(id_i[:, :], pattern=[[1, 128]], base=0, channel_multiplier=-1)
    idf = const.tile([128, 128], f32)
    nc.vector.tensor_scalar(idf[:, :], id_i[:, :], 0, None, mybir.AluOpType.is_equal)
    diag = const.tile([128, KK * KK * 128], bf16)
    for t in range(KK * KK):
        nc.vector.tensor_scalar_mul(
            diag[:, t * 128 : (t + 1) * 128], idf[:, :], kcols[:, t : t + 1]
        )

    # ---------------- main loop ----------------
    xrows = xs[:, :].rearrange("p (r w) -> p r w", w=RS)

    def win(r0, nr, di, dj):
        # image rows r0..r0+nr-1 under tap (di,dj); padded row r0+di covers
        # image row r0+di-1 (the +1 pad offset cancels the tap's -1).
        return xrows[:, r0 + di : r0 + di + nr, dj : dj + W]

    dg = {}  # image row -> (tile, px offset) for 4-row (512 px) slices

    def set_dg(r0, nr, tilev, base=0):
        for i in range(nr // 4):
            dg[r0 + 4 * i] = (tilev, base + 512 * i)

    pcur = {"t": None}

    def pe_group(gi, r0):
        # two groups share one [128,1024] psum tile (2 banks)
        if gi % 2 == 0:
            pcur["t"] = pdw.tile([128, 1024], f32, tag="pdw", name=f"pdw{r0}")
        P = pcur["t"]
        half = 512 * (gi % 2)
        for t, (di, dj) in enumerate(TAPS):
            nc.tensor.matmul(
                P[:, half : half + 512],
                lhsT=diag[:, t * 128 : (t + 1) * 128],
                rhs=win(r0, 4, di, dj),
                start=(t == 0), stop=(t == KK * KK - 1),
            )
        if gi % 2 == 1 or gi == len(PE_GROUPS) - 1:
            npx = half + 512
            rbase = r0 - 4 * (gi % 2)
            D = dgp.tile([128, npx], bf16, name=f"dpe{rbase}")
            nc.scalar.activation(D[:, 0:npx], P[:, 0:npx], LRELU, alpha=0.1)
            set_dg(rbase, npx // 128, D)

    def dve_muls(r0, nr, pool):
        px = nr * W
        prod = pool.tile([128, 9 * px], bf16, tag="prod", name=f"prod{r0}")
        p3 = prod[:, :].rearrange("p (t x) -> p t x", x=px)
        for t, (di, dj) in enumerate(TAPS):
            o = p3[:, t, :].rearrange("p (r w) -> p r w", w=W)
            nc.vector.tensor_scalar_mul(o, win(r0, nr, di, dj), kcols[:, t : t + 1])
        return p3

    def adds_and_lrelu(eng, p3, r0, nr):
        px = nr * W
        for dst, src in ADD_TREE[:-1]:
            eng.tensor_tensor(
                p3[:, dst, :], p3[:, dst, :], p3[:, src, :], op=mybir.AluOpType.add
            )
        acc = accp.tile([128, px], bf16, tag="acc", name=f"acc{r0}")
        eng.tensor_tensor(
            acc[:, :], p3[:, 0, :], p3[:, 8, :], op=mybir.AluOpType.add
        )
        D = dgp.tile([128, px], bf16, name=f"dd{r0}")
        nc.scalar.activation(D[:, :], acc[:, :], LRELU, alpha=0.1)
        set_dg(r0, nr, D)

    # --- 1x1 span (8 rows = 1024 px) + bias evac; out DMA per span pair ---
    ost_tiles = {}
    ost_done = {}

    def span_1x1(s):
        r0 = 8 * s
        O = po2.tile([128, 1024], f32, tag="oo", name=f"o2{s}")
        for h in range(2):
            t_, off = dg[r0 + 4 * h]
            nc.tensor.matmul(
                O[:, 512 * h : 512 * (h + 1)],
                lhsT=wcb[:, :], rhs=t_[:, off : off + 512],
                start=True, stop=True,
            )
        q = s // 2
        if q not in ost_tiles:
            ost_tiles[q] = ostg.tile([128, 2048], bf16, tag="ostg", name=f"ostg{q}")
            ost_done[q] = 0
        z = ost_tiles[q]
        nc.scalar.add(z[:, 1024 * (s % 2) : 1024 * (s % 2 + 1)], O[:, :], bc2[:, 0:1])
        ost_done[q] += 1
        if ost_done[q] == 2:
            nc.sync.dma_start(out_d[:, q * 2048 : (q + 1) * 2048], z[:, :])

    # ---------------- schedule ----------------
    dp_p3 = []
    for r0, nr in DP_CHUNKS:
        dp_p3.append(dve_muls(r0, nr, dpprod))
    for (r0, nr), p3 in zip(DP_CHUNKS, dp_p3):
        adds_and_lrelu(nc.gpsimd, p3, r0, nr)
    for r0, nr in D_CHUNKS:
        p3 = dve_muls(r0, nr, dprod)
        adds_and_lrelu(nc.vector, p3, r0, nr)

    done_rows = set()
    emitted_spans = set()
    for r0, nr in DP_CHUNKS + D_CHUNKS:
        for i in range(nr // 4):
            done_rows.add(r0 + 4 * i)

    def flush_spans():
        for s in range(16):
            if s in emitted_spans:
                continue
            if (8 * s in done_rows) and (8 * s + 4 in done_rows):
                span_1x1(s)
                emitted_spans.add(s)

    for gi, r0 in enumerate(PE_GROUPS):
        pe_group(gi, r0)
        done_rows.add(r0)
        flush_spans()
    flush_spans()
    assert len(emitted_spans) == 16, f"unscheduled spans: {sorted(emitted_spans)}"


# ---------------------------------------------------------------------------
# host-side entry point
# ---------------------------------------------------------------------------

_PROGRAM_CACHE: dict[str, bass.Bass] = {}


def _get_program() -> bass.Bass:
    if "p" not in _PROGRAM_CACHE:
        _PROGRAM_CACHE["p"] = build_program()
    return _PROGRAM_CACHE["p"]


def _host_prep(inputs: dict):
    import ml_dtypes

    x = np.asarray(inputs["x"], dtype=np.float32)
    d = np.asarray(inputs["d"], dtype=np.float32)
    Wk1 = np.asarray(inputs["Wk1"], dtype=np.float32)
    Wk2 = np.asarray(inputs["Wk2"], dtype=np.float32)
    Wc = np.asarray(inputs["Wc"], dtype=np.float32)
    bc = np.asarray(inputs["bc"], dtype=np.float32)

    wk1t = np.ascontiguousarray(Wk1.T)
    w = Wk2.reshape(C, KK * KK, C).transpose(2, 1, 0)  # (j, t, c)
    wk2td = np.ascontiguousarray(
        np.concatenate([w, w], axis=2).reshape(C, KK * KK * 2 * C)
    )
    wcb = np.zeros((2 * C, 2 * C), dtype=np.float32)
    wcb[0:C, 0:C] = Wc.T
    wcb[C:, C:] = Wc.T
    wcb = wcb.astype(ml_dtypes.bfloat16)
    bc2 = np.ascontiguousarray(np.concatenate([bc, bc]).reshape(2 * C, 1))

    # host-side zero-padding: [S*C, RP, RS] with image at [1:H+1, 1:W+1]
    B = x.shape[0]
    xpad = np.zeros((B, C, RP, RS), dtype=ml_dtypes.bfloat16)
    xpad[:, :, 1 : H + 1, 1 : W + 1] = x.astype(ml_dtypes.bfloat16)

    in_maps = []
    for i in range(NCORES):
        xp = np.ascontiguousarray(
            xpad[S * i : S * (i + 1)].reshape(S * C, XFREE)
        )
        dT = np.ascontiguousarray(d[S * i : S * (i + 1)].T)
        in_maps.append(
            {
                "xpad": xp,
                "dT": dT,
                "wk1t": wk1t,
                "wk2td": wk2td,
                "wcb": wcb,
                "bc2": bc2,
            }
        )
    return in_maps


def run_on_hw(inputs: dict, **kwargs):
    """Run the SPMD kernel on 8 NeuronCores; returns (output, results)."""
    from concourse.bass_utils import run_bass_kernel_spmd

    nc = _get_program()
    in_maps = _host_prep(inputs)
    res = run_bass_kernel_spmd(nc, in_maps, core_ids=list(range(NCORES)), **kwargs)
    outs = res.results
    B = S * NCORES
    out = np.empty((B, C, H, W), dtype=np.float32)
    for i in range(NCORES):
        out[S * i : S * (i + 1)] = outs[i]["out"].astype(np.float32).reshape(
            S, C, H, W
        )
    return out, res


def kernel(**inputs) -> np.ndarray:
    out, _ = run_on_hw(inputs)
    return out


if __name__ == "__main__":
    nc = build_program()
    print("program built OK")


# revision 16
# speedup vs baseline: 2.6111x; 1.1683x over previous
"""Trainium2 Bass kernel for nn_DA_conv: per-sample dynamic depthwise 3x3 conv
(+LeakyReLU) followed by a 1x1 pointwise conv, with the 3x3 kernels produced by
a small per-sample MLP.

Strategy (8 NeuronCores, pure batch data-parallel, 2 samples per core):
  - SBUF partition p = (sample s = p//64, channel c = p%64); the 2-sample
    feature map lives resident in SBUF, zero-padded ON THE HOST so the DMA in
    is fully contiguous (1 descriptor per partition per chunk).
  - Depthwise conv split across engines by image-row region:
      * PE rows:  9 PSUM-accumulating full-128-partition diagonal matmuls per
        512-px tile; Prelu evacuation (1024 px) on Act.
      * DP rows:  per-tap products on DVE (tensor_scalar_mul, 4x bf16 mode),
        add tree on Pool (tensor_tensor), LeakyReLU on Act.
      * D rows:   products + add tree fully on DVE, LeakyReLU on Act.
  - 1x1 conv = block-diagonal [128x128] bf16 matmuls; PSUM evacuated by Act
    with the bias add fused (Identity + per-partition bias), written bf16.
  - All DMA transfers are engine-time in this machine model, so x chunks are
    split between the SP and Act queues in consumption order; output DMAs
    ride SP.
"""

import sys

sys.path.insert(0, "/opt/trn_rl_repo")

from contextlib import ExitStack

import numpy as np

import concourse.bacc as bacc
import concourse.bass as bass
import concourse.mybir as mybir
import concourse.tile as tile

S = 2            # samples per core
C = 64           # channels
H = W = 128      # spatial
KK = 3           # conv kernel size
NCORES = 8
RS = 132         # padded row stride in elements
RP = H + 2       # padded row count (top/bottom halo)
XFREE = RP * RS  # padded image elements per partition

f32 = mybir.dt.float32
bf16 = mybir.dt.bfloat16
i32 = mybir.dt.int32

LRELU = mybir.ActivationFunctionType.Prelu
TAPS = [(di, dj) for di in range(KK) for dj in range(KK)]  # t = di*3 + dj

# ---- region assignment (rows of the 128-row image) ----
PE_GROUPS = [4 * g for g in range(17)]              # rows 0..67
DP_CHUNKS = [(68, 4), (72, 12), (84, 12), (96, 8)]  # DVE muls + Pool adds
D_CHUNKS = [(104, 12), (116, 12)]                   # all-DVE
ADD_TREE = [  # (dst, src) pairs over 9 product slots; acc ends in slot 0
    (0, 1), (2, 3), (4, 5), (6, 7), (0, 2), (4, 6), (0, 4), (0, 8),
]
# x chunks in PADDED row space (padded row pr holds image row pr-1), all on
# the SP queue, ordered for earliest consumer.
X_CHUNKS_SP = [(0, 18), (46, 28), (18, 28), (74, 16), (90, 16), (106, 24)]


def build_program() -> bass.Bass:
    nc = bacc.Bacc("TRN2", target_bir_lowering=False, debug=False)

    x_d = nc.dram_tensor("xpad", [S * C, XFREE], bf16, kind="ExternalInput").ap()
    dt_d = nc.dram_tensor("dT", [C, S], bf16, kind="ExternalInput").ap()
    wk1_d = nc.dram_tensor("wk1t", [C, C], bf16, kind="ExternalInput").ap()
    # Wk2 transposed + tap-major + duplicated over samples:
    # wk2td[j, t*128 + s*64 + c] = Wk2[c*9 + t, j]
    wk2_d = nc.dram_tensor("wk2td", [C, KK * KK * 2 * C], bf16, kind="ExternalInput").ap()
    # block-diagonal 1x1 weights: wcb[(s,ci),(s,co)] = Wc[co,ci]
    wcb_d = nc.dram_tensor("wcb", [2 * C, 2 * C], bf16, kind="ExternalInput").ap()
    bc_d = nc.dram_tensor("bc2", [2 * C, 1], f32, kind="ExternalInput").ap()
    out_d = nc.dram_tensor("out", [S * C, H * W], bf16, kind="ExternalOutput").ap()

    with tile.TileContext(nc) as tc, ExitStack() as ctx:
        _body(ctx, tc, x_d, dt_d, wk1_d, wk2_d, wcb_d, bc_d, out_d)
    nc.compile()
    return nc


def _body(ctx, tc, x_d, dt_d, wk1_d, wk2_d, wcb_d, bc_d, out_d):
    nc = tc.nc
    const = ctx.enter_context(tc.tile_pool(name="const", bufs=1))
    xpool = ctx.enter_context(tc.tile_pool(name="xs", bufs=1))
    dgp = ctx.enter_context(tc.tile_pool(name="dg", bufs=1))
    dpprod = ctx.enter_context(tc.tile_pool(name="dpprod", bufs=3))
    dprod = ctx.enter_context(tc.tile_pool(name="dprod", bufs=1))
    accp = ctx.enter_context(tc.tile_pool(name="acc", bufs=2))
    ostg = ctx.enter_context(tc.tile_pool(name="ostg", bufs=4))
    pdw = ctx.enter_context(tc.tile_pool(name="pdw", bufs=2, space="PSUM"))
    po2 = ctx.enter_context(tc.tile_pool(name="po2", bufs=2, space="PSUM"))

    # ---------------- input loads ----------------
    # MLP weights (bf16) on the Act queue first (they gate kcols/diag); all
    # of x plus wcb/bc2 on SP in consumption order.
    dts = const.tile([C, S], bf16)
    nc.scalar.dma_start(dts[:, :], dt_d)
    wk1t = const.tile([C, C], bf16)
    nc.scalar.dma_start(wk1t[:, :], wk1_d)
    wk2td = const.tile([C, KK * KK * 2 * C], bf16)
    nc.scalar.dma_start(wk2td[:, :], wk2_d)

    xs = xpool.tile([128, XFREE], bf16)

    def load_x(engine, pr0, npr):
        engine.dma_start(
            xs[:, pr0 * RS : (pr0 + npr) * RS], x_d[:, pr0 * RS : (pr0 + npr) * RS]
        )

    for chunk in X_CHUNKS_SP[:2]:
        load_x(nc.sync, *chunk)

    # ---------------- kernel-generating MLP ----------------
    hid_ps = po2.tile([C, S], f32, tag="oo")
    nc.tensor.matmul(
        hid_ps[:, :], lhsT=wk1t[:, :], rhs=dts[:, :], start=True, stop=True,
    )
    hid_sb = const.tile([C, S], bf16)
    nc.scalar.activation(hid_sb[:, :], hid_ps[:, :], LRELU, alpha=0.1)

    # kern tap columns: kcols[s*64+c, t] = kern[s, c*9+t].
    # All 9 tap matmuls write one psum tile; two strided copies pick the
    # sample-matched column per partition half.
    kps = po2.tile([2 * C, 2 * KK * KK], f32, tag="oo")
    for t in range(KK * KK):
        nc.tensor.matmul(
            kps[:, 2 * t : 2 * t + 2],
            lhsT=wk2td[:, t * 128 : (t + 1) * 128],
            rhs=hid_sb[:, :],
            start=True, stop=True,
        )
    kcols = const.tile([2 * C, KK * KK], f32)
    kps3 = kps[:, :].rearrange("p (t s) -> p t s", s=2)
    nc.vector.tensor_copy(kcols[0:C, :], kps3[0:C, :, 0])
    nc.vector.tensor_copy(kcols[C : 2 * C, :], kps3[C : 2 * C, :, 1])

    for chunk in X_CHUNKS_SP[2:]:
        load_x(nc.sync, *chunk)
    wcb = const.tile([2 * C, 2 * C], bf16)
    nc.sync.dma_start(wcb[:, :], wcb_d)
    bc2 = const.tile([2 * C, 1], f32)
    nc.sync.dma_start(bc2[:, :], bc_d)

    # identity -> per-tap diagonal weight matrices diag[:, t*128:(t+1)*128]
    id_i = const.tile([128, 128], i32)
    nc.gpsimd.iota(id_i[:, :], pattern=[[1, 128]], base=0, channel_multiplier=-1)
    idf = const.tile([128, 128], f32)
    nc.vector.tensor_scalar(idf[:, :], id_i[:, :], 0, None, mybir.AluOpType.is_equal)
    diag = const.tile([128, KK * KK * 128], bf16)
    for t in range(KK * KK):
        nc.vector.tensor_scalar_mul(
            diag[:, t * 128 : (t + 1) * 128], idf[:, :], kcols[:, t : t + 1]
        )

    # ---------------- main loop ----------------
    xrows = xs[:, :].rearrange("p (r w) -> p r w", w=RS)

    def win(r0, nr, di, dj):
        # image rows r0..r0+nr-1 under tap (di,dj); padded row r0+di covers
        # image row r0+di-1 (the +1 pad offset cancels the tap's -1).
        return xrows[:, r0 + di : r0 + di + nr, dj : dj + W]

    dg = {}  # image row -> (tile, px offset) for 4-row (512 px) slices

    def set_dg(r0, nr, tilev, base=0):
        for i in range(nr // 4):
            dg[r0 + 4 * i] = (tilev, base + 512 * i)

    pcur = {"t": None}

    def pe_group(gi, r0):
        # two groups share one [128,1024] psum tile (2 banks)
        if gi % 2 == 0:
            pcur["t"] = pdw.tile([128, 1024], f32, tag="pdw", name=f"pdw{r0}")
        P = pcur["t"]
        half = 512 * (gi % 2)
        for t, (di, dj) in enumerate(TAPS):
            nc.tensor.matmul(
                P[:, half : half + 512],
                lhsT=diag[:, t * 128 : (t + 1) * 128],
                rhs=win(r0, 4, di, dj),
                start=(t == 0), stop=(t == KK * KK - 1),
            )
        if gi % 2 == 1 or gi == len(PE_GROUPS) - 1:
            npx = half + 512
            rbase = r0 - 4 * (gi % 2)
            D = dgp.tile([128, npx], bf16, name=f"dpe{rbase}")
            nc.scalar.activation(D[:, 0:npx], P[:, 0:npx], LRELU, alpha=0.1)
            set_dg(rbase, npx // 128, D)

    def dve_muls(r0, nr, pool):
        px = nr * W
        prod = pool.tile([128, 9 * px], bf16, tag="prod", name=f"prod{r0}")
        p3 = prod[:, :].rearrange("p (t x) -> p t x", x=px)
        for t, (di, dj) in enumerate(TAPS):
            o = p3[:, t, :].rearrange("p (r w) -> p r w", w=W)
            nc.vector.tensor_scalar_mul(o, win(r0, nr, di, dj), kcols[:, t : t + 1])
        return p3

    def adds_and_lrelu(eng, p3, r0, nr, tag):
        px = nr * W
        for dst, src in ADD_TREE[:-1]:
            eng.tensor_tensor(
                p3[:, dst, :], p3[:, dst, :], p3[:, src, :], op=mybir.AluOpType.add
            )
        acc = accp.tile([128, px], bf16, tag=tag, name=f"acc{r0}")
        eng.tensor_tensor(
            acc[:, :], p3[:, 0, :], p3[:, 8, :], op=mybir.AluOpType.add
        )
        D = dgp.tile([128, px], bf16, name=f"dd{r0}")
        nc.scalar.activation(D[:, :], acc[:, :], LRELU, alpha=0.1)
        set_dg(r0, nr, D)

    # --- 1x1 span (8 rows = 1024 px) + bias evac; out DMA per span pair ---
    ost_tiles = {}
    ost_done = {}

    def span_1x1(s):
        r0 = 8 * s
        O = po2.tile([128, 1024], f32, tag="oo", name=f"o2{s}")
        for h in range(2):
            t_, off = dg[r0 + 4 * h]
            nc.tensor.matmul(
                O[:, 512 * h : 512 * (h + 1)],
                lhsT=wcb[:, :], rhs=t_[:, off : off + 512],
                start=True, stop=True,
            )
        q = s // 2
        if q not in ost_tiles:
            ost_tiles[q] = ostg.tile([128, 2048], bf16, tag="ostg", name=f"ostg{q}")
            ost_done[q] = 0
        z = ost_tiles[q]
        nc.scalar.add(z[:, 1024 * (s % 2) : 1024 * (s % 2 + 1)], O[:, :], bc2[:, 0:1])
        ost_done[q] += 1
        if ost_done[q] == 2:
            nc.sync.dma_start(out_d[:, q * 2048 : (q + 1) * 2048], z[:, :])

    # ---------------- schedule (virtual-time ordered emission) ----------
    # Engines execute their streams near-order with a small lookahead, so
    # emit every op at its estimated ready time to avoid head-of-line
    # convoys.  Costs in us, from the TRN2 cost model.
    MUL_C = lambda px: (px * 0.268 + 105) / 1000.0
    ADD_C = lambda px: (px * 0.53 + 105) / 1000.0
    PADD_C = lambda px: (px * 0.833 + 131) / 1000.0

    events = []  # (vtime, seq, emit_fn)
    seq = [0]

    def ev(vt, fn):
        events.append((vt, seq[0], fn))
        seq[0] += 1

    row_ready = {}  # image row (mult of 4) -> vtime its D tile is ready

    # PE dw groups: start ~5.0, ~1.94us each; prelu lands with the pair.
    vt = 5.0
    for gi, r0 in enumerate(PE_GROUPS):
        vt += 1.94
        ev(vt, (lambda gi=gi, r0=r0: pe_group(gi, r0)))
        if gi % 2 == 1 or gi == len(PE_GROUPS) - 1:
            rbase = r0 - 4 * (gi % 2)
            for rr in range(rbase, r0 + 4, 4):
                row_ready[rr] = vt + 0.9

    # DVE: DP muls first, then D chunks (muls+adds).  DVE clock starts ~5.
    dvt = 5.0
    for r0, nr in DP_CHUNKS:
        dvt += 9 * MUL_C(nr * W)
        ev(dvt - 9 * MUL_C(nr * W),
           (lambda r0=r0, nr=nr: dp_p3.__setitem__(r0, dve_muls(r0, nr, dpprod))))
    dp_mul_done = {}
    dvt2 = 5.0
    for r0, nr in DP_CHUNKS:
        dvt2 += 9 * MUL_C(nr * W)
        dp_mul_done[r0] = dvt2
    for r0, nr in D_CHUNKS:
        cost = 9 * MUL_C(nr * W) + 8 * ADD_C(nr * W)
        ev(dvt, (lambda r0=r0, nr=nr: d_chunk(r0, nr)))
        dvt += cost
        for rr in range(r0, r0 + nr, 4):
            row_ready[rr] = dvt + 1.2

    # Pool: add trees, serial, gated by the DP muls.
    pvt = 0.0
    for r0, nr in DP_CHUNKS:
        pvt = max(pvt, dp_mul_done[r0])
        ev(pvt, (lambda r0=r0, nr=nr: dp_adds(r0, nr)))
        pvt += 8 * PADD_C(nr * W)
        for rr in range(r0, r0 + nr, 4):
            row_ready[rr] = pvt + 1.2

    # 1x1 spans at max over their two D tiles' readiness.
    for s in range(16):
        rt = max(row_ready[8 * s], row_ready[8 * s + 4])
        ev(rt, (lambda s=s: span_1x1(s)))

    dp_p3 = {}

    def dp_adds(r0, nr):
        adds_and_lrelu(nc.gpsimd, dp_p3[r0], r0, nr, "pacc")

    def d_chunk(r0, nr):
        p3 = dve_muls(r0, nr, dprod)
        adds_and_lrelu(nc.vector, p3, r0, nr, "dacc")

    for _, _, fn in sorted(events, key=lambda e: (e[0], e[1])):
        fn()


# ---------------------------------------------------------------------------
# host-side entry point
# ---------------------------------------------------------------------------

_PROGRAM_CACHE: dict[str, bass.Bass] = {}


def _get_program() -> bass.Bass:
    if "p" not in _PROGRAM_CACHE:
        _PROGRAM_CACHE["p"] = build_program()
    return _PROGRAM_CACHE["p"]


def _host_prep(inputs: dict):
    import ml_dtypes

    x = np.asarray(inputs["x"], dtype=np.float32)
    d = np.asarray(inputs["d"], dtype=np.float32)
    Wk1 = np.asarray(inputs["Wk1"], dtype=np.float32)
    Wk2 = np.asarray(inputs["Wk2"], dtype=np.float32)
    Wc = np.asarray(inputs["Wc"], dtype=np.float32)
    bc = np.asarray(inputs["bc"], dtype=np.float32)

    wk1t = np.ascontiguousarray(Wk1.T).astype(ml_dtypes.bfloat16)
    w = Wk2.reshape(C, KK * KK, C).transpose(2, 1, 0)  # (j, t, c)
    wk2td = np.ascontiguousarray(
        np.concatenate([w, w], axis=2).reshape(C, KK * KK * 2 * C)
    ).astype(ml_dtypes.bfloat16)
    wcb = np.zeros((2 * C, 2 * C), dtype=np.float32)
    wcb[0:C, 0:C] = Wc.T
    wcb[C:, C:] = Wc.T
    wcb = wcb.astype(ml_dtypes.bfloat16)
    bc2 = np.ascontiguousarray(np.concatenate([bc, bc]).reshape(2 * C, 1))

    # host-side zero-padding: [S*C, RP, RS] with image at [1:H+1, 1:W+1]
    B = x.shape[0]
    xpad = np.zeros((B, C, RP, RS), dtype=ml_dtypes.bfloat16)
    xpad[:, :, 1 : H + 1, 1 : W + 1] = x.astype(ml_dtypes.bfloat16)

    in_maps = []
    for i in range(NCORES):
        xp = np.ascontiguousarray(
            xpad[S * i : S * (i + 1)].reshape(S * C, XFREE)
        )
        dT = np.ascontiguousarray(d[S * i : S * (i + 1)].T).astype(ml_dtypes.bfloat16)
        in_maps.append(
            {
                "xpad": xp,
                "dT": dT,
                "wk1t": wk1t,
                "wk2td": wk2td,
                "wcb": wcb,
                "bc2": bc2,
            }
        )
    return in_maps


def run_on_hw(inputs: dict, **kwargs):
    """Run the SPMD kernel on 8 NeuronCores; returns (output, results)."""
    from concourse.bass_utils import run_bass_kernel_spmd

    nc = _get_program()
    in_maps = _host_prep(inputs)
    res = run_bass_kernel_spmd(nc, in_maps, core_ids=list(range(NCORES)), **kwargs)
    outs = res.results
    B = S * NCORES
    out = np.empty((B, C, H, W), dtype=np.float32)
    for i in range(NCORES):
        out[S * i : S * (i + 1)] = outs[i]["out"].astype(np.float32).reshape(
            S, C, H, W
        )
    return out, res


def kernel(**inputs) -> np.ndarray:
    out, _ = run_on_hw(inputs)
    return out


if __name__ == "__main__":
    nc = build_program()
    print("program built OK")


# revision 21
# speedup vs baseline: 2.6586x; 1.0182x over previous
"""Trainium2 Bass kernel for nn_DA_conv: per-sample dynamic depthwise 3x3 conv
(+LeakyReLU) followed by a 1x1 pointwise conv, with the 3x3 kernels produced by
a small per-sample MLP.

Strategy (8 NeuronCores, pure batch data-parallel, 2 samples per core):
  - SBUF partition p = (sample s = p//64, channel c = p%64); the 2-sample
    feature map lives resident in SBUF, zero-padded ON THE HOST so the DMA in
    is fully contiguous (1 descriptor per partition per chunk).
  - Depthwise conv split across engines by image-row region:
      * PE rows:  9 PSUM-accumulating full-128-partition diagonal matmuls per
        512-px tile; Prelu evacuation (1024 px) on Act.
      * DP rows:  per-tap products on DVE (tensor_scalar_mul, 4x bf16 mode),
        add tree on Pool (tensor_tensor), LeakyReLU on Act.
      * D rows:   products + add tree fully on DVE, LeakyReLU on Act.
  - 1x1 conv = block-diagonal [128x128] bf16 matmuls; PSUM evacuated by Act
    with the bias add fused (Identity + per-partition bias), written bf16.
  - All DMA transfers are engine-time in this machine model, so x chunks are
    split between the SP and Act queues in consumption order; output DMAs
    ride SP.
"""

import sys

sys.path.insert(0, "/opt/trn_rl_repo")

from contextlib import ExitStack

import numpy as np

import concourse.bacc as bacc
import concourse.bass as bass
import concourse.mybir as mybir
import concourse.tile as tile

S = 2            # samples per core
C = 64           # channels
H = W = 128      # spatial
KK = 3           # conv kernel size
NCORES = 8
RS = 132         # padded row stride in elements
RP = H + 2       # padded row count (top/bottom halo)
XFREE = RP * RS  # padded image elements per partition

f32 = mybir.dt.float32
bf16 = mybir.dt.bfloat16
i32 = mybir.dt.int32

LRELU = mybir.ActivationFunctionType.Prelu
TAPS = [(di, dj) for di in range(KK) for dj in range(KK)]  # t = di*3 + dj

# ---- region assignment (rows of the 128-row image) ----
PE_GROUPS = [4 * g for g in range(17)]              # rows 0..67
DP_CHUNKS = [(68, 4), (72, 12), (84, 12), (96, 8)]  # DVE muls + Pool adds
D_CHUNKS = [(104, 12), (116, 12)]                   # all-DVE
ADD_TREE = [  # (dst, src) pairs over 9 product slots; acc ends in slot 0
    (0, 1), (2, 3), (4, 5), (6, 7), (0, 2), (4, 6), (0, 4), (0, 8),
]
# x chunks in PADDED row space (padded row pr holds image row pr-1), all on
# the SP queue, ordered for earliest consumer.
X_CHUNKS_SP = [(0, 18), (46, 28), (74, 16), (18, 28), (90, 16), (106, 24)]


def build_program() -> bass.Bass:
    nc = bacc.Bacc("TRN2", target_bir_lowering=False, debug=False)

    x_d = nc.dram_tensor("xpad", [S * C, XFREE], bf16, kind="ExternalInput").ap()
    dt_d = nc.dram_tensor("dT", [C, S], bf16, kind="ExternalInput").ap()
    wk1_d = nc.dram_tensor("wk1t", [C, C], bf16, kind="ExternalInput").ap()
    # Wk2 transposed + tap-major + duplicated over samples:
    # wk2td[j, t*128 + s*64 + c] = Wk2[c*9 + t, j]
    wk2_d = nc.dram_tensor("wk2td", [C, KK * KK * 2 * C], bf16, kind="ExternalInput").ap()
    # block-diagonal 1x1 weights: wcb[(s,ci),(s,co)] = Wc[co,ci]
    wcb_d = nc.dram_tensor("wcb", [2 * C, 2 * C], bf16, kind="ExternalInput").ap()
    bc_d = nc.dram_tensor("bc2", [2 * C, 1], f32, kind="ExternalInput").ap()
    out_d = nc.dram_tensor("out", [S * C, H * W], bf16, kind="ExternalOutput").ap()

    with tile.TileContext(nc) as tc, ExitStack() as ctx:
        _body(ctx, tc, x_d, dt_d, wk1_d, wk2_d, wcb_d, bc_d, out_d)
    nc.compile()
    return nc


def _body(ctx, tc, x_d, dt_d, wk1_d, wk2_d, wcb_d, bc_d, out_d):
    nc = tc.nc
    const = ctx.enter_context(tc.tile_pool(name="const", bufs=1))
    xpool = ctx.enter_context(tc.tile_pool(name="xs", bufs=1))
    dgp = ctx.enter_context(tc.tile_pool(name="dg", bufs=1))
    dpprod = ctx.enter_context(tc.tile_pool(name="dpprod", bufs=3))
    dprod = ctx.enter_context(tc.tile_pool(name="dprod", bufs=1))
    accp = ctx.enter_context(tc.tile_pool(name="acc", bufs=2))
    ostg = ctx.enter_context(tc.tile_pool(name="ostg", bufs=4))
    pdw = ctx.enter_context(tc.tile_pool(name="pdw", bufs=2, space="PSUM"))
    po2 = ctx.enter_context(tc.tile_pool(name="po2", bufs=2, space="PSUM"))

    # ---------------- input loads ----------------
    # MLP weights (bf16) on the Act queue first (they gate kcols/diag); all
    # of x plus wcb/bc2 on SP in consumption order.
    dts = const.tile([C, S], bf16)
    nc.scalar.dma_start(dts[:, :], dt_d)
    wk1t = const.tile([C, C], bf16)
    nc.scalar.dma_start(wk1t[:, :], wk1_d)
    wk2td = const.tile([C, KK * KK * 2 * C], bf16)
    nc.scalar.dma_start(wk2td[:, :], wk2_d)

    xs = xpool.tile([128, XFREE], bf16)

    def load_x(engine, pr0, npr):
        engine.dma_start(
            xs[:, pr0 * RS : (pr0 + npr) * RS], x_d[:, pr0 * RS : (pr0 + npr) * RS]
        )

    for chunk in X_CHUNKS_SP[:2]:
        load_x(nc.sync, *chunk)

    # ---------------- kernel-generating MLP ----------------
    hid_ps = po2.tile([C, S], f32, tag="oo")
    nc.tensor.matmul(
        hid_ps[:, :], lhsT=wk1t[:, :], rhs=dts[:, :], start=True, stop=True,
    )
    hid_sb = const.tile([C, S], bf16)
    nc.scalar.activation(hid_sb[:, :], hid_ps[:, :], LRELU, alpha=0.1)

    # kern tap columns: kcols[s*64+c, t] = kern[s, c*9+t].
    # All 9 tap matmuls write one psum tile; two strided copies pick the
    # sample-matched column per partition half.
    kps = po2.tile([2 * C, 2 * KK * KK], f32, tag="oo")
    for t in range(KK * KK):
        nc.tensor.matmul(
            kps[:, 2 * t : 2 * t + 2],
            lhsT=wk2td[:, t * 128 : (t + 1) * 128],
            rhs=hid_sb[:, :],
            start=True, stop=True,
        )
    kcols = const.tile([2 * C, KK * KK], f32)
    kps3 = kps[:, :].rearrange("p (t s) -> p t s", s=2)
    nc.vector.tensor_copy(kcols[0:C, :], kps3[0:C, :, 0])
    nc.vector.tensor_copy(kcols[C : 2 * C, :], kps3[C : 2 * C, :, 1])

    for chunk in X_CHUNKS_SP[2:]:
        load_x(nc.sync, *chunk)
    wcb = const.tile([2 * C, 2 * C], bf16)
    nc.sync.dma_start(wcb[:, :], wcb_d)
    bc2 = const.tile([2 * C, 1], f32)
    nc.sync.dma_start(bc2[:, :], bc_d)

    # identity -> per-tap diagonal weight matrices diag[:, t*128:(t+1)*128]
    id_i = const.tile([128, 128], i32)
    nc.gpsimd.iota(id_i[:, :], pattern=[[1, 128]], base=0, channel_multiplier=-1)
    idf = const.tile([128, 128], f32)
    nc.vector.tensor_scalar(idf[:, :], id_i[:, :], 0, None, mybir.AluOpType.is_equal)
    diag = const.tile([128, KK * KK * 128], bf16)
    for t in range(KK * KK):
        nc.vector.tensor_scalar_mul(
            diag[:, t * 128 : (t + 1) * 128], idf[:, :], kcols[:, t : t + 1]
        )

    # ---------------- main loop ----------------
    xrows = xs[:, :].rearrange("p (r w) -> p r w", w=RS)

    def win(r0, nr, di, dj):
        # image rows r0..r0+nr-1 under tap (di,dj); padded row r0+di covers
        # image row r0+di-1 (the +1 pad offset cancels the tap's -1).
        return xrows[:, r0 + di : r0 + di + nr, dj : dj + W]

    dg = {}  # image row -> (tile, px offset) for 4-row (512 px) slices

    def set_dg(r0, nr, tilev, base=0):
        for i in range(nr // 4):
            dg[r0 + 4 * i] = (tilev, base + 512 * i)

    pcur = {"t": None}

    def pe_group(gi, r0):
        # two groups share one [128,1024] psum tile (2 banks)
        if gi % 2 == 0:
            pcur["t"] = pdw.tile([128, 1024], f32, tag="pdw", name=f"pdw{r0}")
        P = pcur["t"]
        half = 512 * (gi % 2)
        for t, (di, dj) in enumerate(TAPS):
            nc.tensor.matmul(
                P[:, half : half + 512],
                lhsT=diag[:, t * 128 : (t + 1) * 128],
                rhs=win(r0, 4, di, dj),
                start=(t == 0), stop=(t == KK * KK - 1),
            )
        if gi % 2 == 1 or gi == len(PE_GROUPS) - 1:
            npx = half + 512
            rbase = r0 - 4 * (gi % 2)
            D = dgp.tile([128, npx], bf16, name=f"dpe{rbase}")
            nc.scalar.activation(D[:, 0:npx], P[:, 0:npx], LRELU, alpha=0.1)
            set_dg(rbase, npx // 128, D)

    def dve_muls(r0, nr, pool):
        px = nr * W
        prod = pool.tile([128, 9 * px], bf16, tag="prod", name=f"prod{r0}")
        p3 = prod[:, :].rearrange("p (t x) -> p t x", x=px)
        for t, (di, dj) in enumerate(TAPS):
            o = p3[:, t, :].rearrange("p (r w) -> p r w", w=W)
            nc.vector.tensor_scalar_mul(o, win(r0, nr, di, dj), kcols[:, t : t + 1])
        return p3

    def adds_and_lrelu(eng, p3, r0, nr, tag, lrelu_dve=False):
        px = nr * W
        for dst, src in ADD_TREE[:-1]:
            eng.tensor_tensor(
                p3[:, dst, :], p3[:, dst, :], p3[:, src, :], op=mybir.AluOpType.add
            )
        acc = accp.tile([128, px], bf16, tag=tag, name=f"acc{r0}")
        eng.tensor_tensor(
            acc[:, :], p3[:, 0, :], p3[:, 8, :], op=mybir.AluOpType.add
        )
        D = dgp.tile([128, px], bf16, name=f"dd{r0}")
        if lrelu_dve:
            # lrelu(v) = max(v, 0.1v) on DVE keeps the chain on one engine
            nc.vector.scalar_tensor_tensor(
                D[:, :], acc[:, :], 0.1, acc[:, :],
                op0=mybir.AluOpType.mult, op1=mybir.AluOpType.max,
            )
        else:
            nc.scalar.activation(D[:, :], acc[:, :], LRELU, alpha=0.1)
        set_dg(r0, nr, D)

    # --- 1x1 span (8 rows = 1024 px) + bias evac; out DMA per span pair ---
    ost_tiles = {}
    ost_done = {}

    def span_1x1(s, evac_dve=False, out_pool=False):
        r0 = 8 * s
        O = po2.tile([128, 1024], f32, tag="oo", name=f"o2{s}")
        for h in range(2):
            t_, off = dg[r0 + 4 * h]
            nc.tensor.matmul(
                O[:, 512 * h : 512 * (h + 1)],
                lhsT=wcb[:, :], rhs=t_[:, off : off + 512],
                start=True, stop=True,
            )
        q = s // 2
        if q not in ost_tiles:
            ost_tiles[q] = ostg.tile([128, 2048], bf16, tag="ostg", name=f"ostg{q}")
            ost_done[q] = 0
        z = ost_tiles[q]
        zsl = z[:, 1024 * (s % 2) : 1024 * (s % 2 + 1)]
        if evac_dve:
            nc.vector.tensor_scalar_add(zsl, O[:, :], bc2[:, 0:1])
        else:
            nc.scalar.add(zsl, O[:, :], bc2[:, 0:1])
        ost_done[q] += 1
        if ost_done[q] == 2:
            eng = nc.gpsimd if out_pool else nc.sync
            eng.dma_start(out_d[:, q * 2048 : (q + 1) * 2048], z[:, :])

    # ---------------- schedule (virtual-time ordered emission) ----------
    # Engines execute their streams near-order with a small lookahead, so
    # emit every op at its estimated ready time to avoid head-of-line
    # convoys.  Costs in us, from the TRN2 cost model.
    MUL_C = lambda px: (px * 0.268 + 105) / 1000.0
    ADD_C = lambda px: (px * 0.53 + 105) / 1000.0
    PADD_C = lambda px: (px * 0.833 + 131) / 1000.0

    events = []  # (vtime, seq, emit_fn)
    seq = [0]

    def ev(vt, fn):
        events.append((vt, seq[0], fn))
        seq[0] += 1

    row_ready = {}  # image row (mult of 4) -> vtime its D tile is ready

    # PE dw groups: start ~5.0, ~1.94us each; prelu lands with the pair.
    vt = 5.0
    for gi, r0 in enumerate(PE_GROUPS):
        vt += 1.94
        ev(vt, (lambda gi=gi, r0=r0: pe_group(gi, r0)))
        if gi % 2 == 1 or gi == len(PE_GROUPS) - 1:
            rbase = r0 - 4 * (gi % 2)
            for rr in range(rbase, r0 + 4, 4):
                row_ready[rr] = vt + 0.9

    # DVE: DP muls first, then D chunks (muls+adds).  DVE clock starts ~5.
    dvt = 5.0
    for r0, nr in DP_CHUNKS:
        dvt += 9 * MUL_C(nr * W)
        ev(dvt - 9 * MUL_C(nr * W),
           (lambda r0=r0, nr=nr: dp_p3.__setitem__(r0, dve_muls(r0, nr, dpprod))))
    dp_mul_done = {}
    dvt2 = 5.0
    for r0, nr in DP_CHUNKS:
        dvt2 += 9 * MUL_C(nr * W)
        dp_mul_done[r0] = dvt2
    for ci, (r0, nr) in enumerate(D_CHUNKS):
        cost = 9 * MUL_C(nr * W) + 8 * ADD_C(nr * W)
        last = ci == len(D_CHUNKS) - 1
        ev(dvt, (lambda r0=r0, nr=nr, last=last: d_chunk(r0, nr, last)))
        dvt += cost
        for rr in range(r0, r0 + nr, 4):
            row_ready[rr] = dvt + 1.2

    # Pool: add trees, serial, gated by the DP muls.
    pvt = 0.0
    for r0, nr in DP_CHUNKS:
        pvt = max(pvt, dp_mul_done[r0])
        ev(pvt, (lambda r0=r0, nr=nr: dp_adds(r0, nr)))
        pvt += 8 * PADD_C(nr * W)
        for rr in range(r0, r0 + nr, 4):
            row_ready[rr] = pvt + 1.2

    # 1x1 spans at max over their two D tiles' readiness.  Late spans use
    # DVE for the bias evac (Act is the convoy then) and the Pool DMA queue
    # for the final output pairs.
    for s in range(16):
        rt = max(row_ready[8 * s], row_ready[8 * s + 4])
        ev(rt, (lambda s=s, rt=rt: span_1x1(s, evac_dve=(rt > 34.0),
                                            out_pool=(rt > 34.0))))

    dp_p3 = {}

    def dp_adds(r0, nr):
        adds_and_lrelu(nc.gpsimd, dp_p3[r0], r0, nr, "pacc")

    def d_chunk(r0, nr, last=False):
        p3 = dve_muls(r0, nr, dprod)
        adds_and_lrelu(nc.vector, p3, r0, nr, "dacc", lrelu_dve=last)

    for _, _, fn in sorted(events, key=lambda e: (e[0], e[1])):
        fn()


# ---------------------------------------------------------------------------
# host-side entry point
# ---------------------------------------------------------------------------

_PROGRAM_CACHE: dict[str, bass.Bass] = {}


def _get_program() -> bass.Bass:
    if "p" not in _PROGRAM_CACHE:
        _PROGRAM_CACHE["p"] = build_program()
    return _PROGRAM_CACHE["p"]


def _host_prep(inputs: dict):
    import ml_dtypes

    x = np.asarray(inputs["x"], dtype=np.float32)
    d = np.asarray(inputs["d"], dtype=np.float32)
    Wk1 = np.asarray(inputs["Wk1"], dtype=np.float32)
    Wk2 = np.asarray(inputs["Wk2"], dtype=np.float32)
    Wc = np.asarray(inputs["Wc"], dtype=np.float32)
    bc = np.asarray(inputs["bc"], dtype=np.float32)

    wk1t = np.ascontiguousarray(Wk1.T).astype(ml_dtypes.bfloat16)
    w = Wk2.reshape(C, KK * KK, C).transpose(2, 1, 0)  # (j, t, c)
    wk2td = np.ascontiguousarray(
        np.concatenate([w, w], axis=2).reshape(C, KK * KK * 2 * C)
    ).astype(ml_dtypes.bfloat16)
    wcb = np.zeros((2 * C, 2 * C), dtype=np.float32)
    wcb[0:C, 0:C] = Wc.T
    wcb[C:, C:] = Wc.T
    wcb = wcb.astype(ml_dtypes.bfloat16)
    bc2 = np.ascontiguousarray(np.concatenate([bc, bc]).reshape(2 * C, 1))

    # host-side zero-padding: [S*C, RP, RS] with image at [1:H+1, 1:W+1]
    B = x.shape[0]
    xpad = np.zeros((B, C, RP, RS), dtype=ml_dtypes.bfloat16)
    xpad[:, :, 1 : H + 1, 1 : W + 1] = x.astype(ml_dtypes.bfloat16)

    in_maps = []
    for i in range(NCORES):
        xp = np.ascontiguousarray(
            xpad[S * i : S * (i + 1)].reshape(S * C, XFREE)
        )
        dT = np.ascontiguousarray(d[S * i : S * (i + 1)].T).astype(ml_dtypes.bfloat16)
        in_maps.append(
            {
                "xpad": xp,
                "dT": dT,
                "wk1t": wk1t,
                "wk2td": wk2td,
                "wcb": wcb,
                "bc2": bc2,
            }
        )
    return in_maps


def run_on_hw(inputs: dict, **kwargs):
    """Run the SPMD kernel on 8 NeuronCores; returns (output, results)."""
    from concourse.bass_utils import run_bass_kernel_spmd

    nc = _get_program()
    in_maps = _host_prep(inputs)
    res = run_bass_kernel_spmd(nc, in_maps, core_ids=list(range(NCORES)), **kwargs)
    outs = res.results
    B = S * NCORES
    out = np.empty((B, C, H, W), dtype=np.float32)
    for i in range(NCORES):
        out[S * i : S * (i + 1)] = outs[i]["out"].astype(np.float32).reshape(
            S, C, H, W
        )
    return out, res


def kernel(**inputs) -> np.ndarray:
    out, _ = run_on_hw(inputs)
    return out


if __name__ == "__main__":
    nc = build_program()
    print("program built OK")


# revision 22
# speedup vs baseline: 2.7104x; 1.0195x over previous
"""Trainium2 Bass kernel for nn_DA_conv: per-sample dynamic depthwise 3x3 conv
(+LeakyReLU) followed by a 1x1 pointwise conv, with the 3x3 kernels produced by
a small per-sample MLP.

Strategy (8 NeuronCores, pure batch data-parallel, 2 samples per core):
  - SBUF partition p = (sample s = p//64, channel c = p%64); the 2-sample
    feature map lives resident in SBUF, zero-padded ON THE HOST so the DMA in
    is fully contiguous (1 descriptor per partition per chunk).
  - Depthwise conv split across engines by image-row region:
      * PE rows:  9 PSUM-accumulating full-128-partition diagonal matmuls per
        512-px tile; Prelu evacuation (1024 px) on Act.
      * DP rows:  per-tap products on DVE (tensor_scalar_mul, 4x bf16 mode),
        add tree on Pool (tensor_tensor), LeakyReLU on Act.
      * D rows:   products + add tree fully on DVE, LeakyReLU on Act.
  - 1x1 conv = block-diagonal [128x128] bf16 matmuls; PSUM evacuated by Act
    with the bias add fused (Identity + per-partition bias), written bf16.
  - All DMA transfers are engine-time in this machine model, so x chunks are
    split between the SP and Act queues in consumption order; output DMAs
    ride SP.
"""

import sys

sys.path.insert(0, "/opt/trn_rl_repo")

from contextlib import ExitStack

import numpy as np

import concourse.bacc as bacc
import concourse.bass as bass
import concourse.mybir as mybir
import concourse.tile as tile

S = 2            # samples per core
C = 64           # channels
H = W = 128      # spatial
KK = 3           # conv kernel size
NCORES = 8
RS = 132         # padded row stride in elements
RP = H + 2       # padded row count (top/bottom halo)
XFREE = RP * RS  # padded image elements per partition

f32 = mybir.dt.float32
bf16 = mybir.dt.bfloat16
i32 = mybir.dt.int32

LRELU = mybir.ActivationFunctionType.Prelu
TAPS = [(di, dj) for di in range(KK) for dj in range(KK)]  # t = di*3 + dj

# ---- region assignment (rows of the 128-row image) ----
PE_GROUPS = [4 * g for g in range(17)]              # rows 0..67
DP_CHUNKS = [(68, 4), (72, 12), (84, 12), (96, 8)]  # DVE muls + Pool adds
D_CHUNKS = [(104, 12), (116, 12)]                   # all-DVE
ADD_TREE = [  # (dst, src) pairs over 9 product slots; acc ends in slot 0
    (0, 1), (2, 3), (4, 5), (6, 7), (0, 2), (4, 6), (0, 4), (0, 8),
]
# x chunks in PADDED row space (padded row pr holds image row pr-1), all on
# the SP queue, ordered for earliest consumer.
X_CHUNKS_SP = [(0, 18), (46, 28), (74, 16), (18, 28), (90, 16), (106, 24)]


def build_program() -> bass.Bass:
    nc = bacc.Bacc("TRN2", target_bir_lowering=False, debug=False)

    x_d = nc.dram_tensor("xpad", [S * C, XFREE], bf16, kind="ExternalInput").ap()
    dt_d = nc.dram_tensor("dT", [C, S], bf16, kind="ExternalInput").ap()
    wk1_d = nc.dram_tensor("wk1t", [C, C], bf16, kind="ExternalInput").ap()
    # Wk2 transposed + tap-major + duplicated over samples:
    # wk2td[j, t*128 + s*64 + c] = Wk2[c*9 + t, j]
    wk2_d = nc.dram_tensor("wk2td", [C, KK * KK * 2 * C], bf16, kind="ExternalInput").ap()
    # block-diagonal 1x1 weights: wcb[(s,ci),(s,co)] = Wc[co,ci]
    wcb_d = nc.dram_tensor("wcb", [2 * C, 2 * C], bf16, kind="ExternalInput").ap()
    bc_d = nc.dram_tensor("bc2", [2 * C, 1], f32, kind="ExternalInput").ap()
    out_d = nc.dram_tensor("out", [S * C, H * W], bf16, kind="ExternalOutput").ap()

    with tile.TileContext(nc) as tc, ExitStack() as ctx:
        _body(ctx, tc, x_d, dt_d, wk1_d, wk2_d, wcb_d, bc_d, out_d)
    nc.compile()
    return nc


def _body(ctx, tc, x_d, dt_d, wk1_d, wk2_d, wcb_d, bc_d, out_d):
    nc = tc.nc
    const = ctx.enter_context(tc.tile_pool(name="const", bufs=1))
    xpool = ctx.enter_context(tc.tile_pool(name="xs", bufs=1))
    dgp = ctx.enter_context(tc.tile_pool(name="dg", bufs=1))
    dpprod = ctx.enter_context(tc.tile_pool(name="dpprod", bufs=3))
    dprod = ctx.enter_context(tc.tile_pool(name="dprod", bufs=1))
    accp = ctx.enter_context(tc.tile_pool(name="acc", bufs=2))
    ostg = ctx.enter_context(tc.tile_pool(name="ostg", bufs=4))
    pdw = ctx.enter_context(tc.tile_pool(name="pdw", bufs=2, space="PSUM"))
    po2 = ctx.enter_context(tc.tile_pool(name="po2", bufs=2, space="PSUM"))

    # ---------------- input loads ----------------
    # MLP weights (bf16) on the Act queue first (they gate kcols/diag); all
    # of x plus wcb/bc2 on SP in consumption order.
    dts = const.tile([C, S], bf16)
    nc.scalar.dma_start(dts[:, :], dt_d)
    wk1t = const.tile([C, C], bf16)
    nc.scalar.dma_start(wk1t[:, :], wk1_d)
    wk2td = const.tile([C, KK * KK * 2 * C], bf16)
    nc.scalar.dma_start(wk2td[:, :], wk2_d)

    xs = xpool.tile([128, XFREE], bf16)

    def load_x(engine, pr0, npr):
        engine.dma_start(
            xs[:, pr0 * RS : (pr0 + npr) * RS], x_d[:, pr0 * RS : (pr0 + npr) * RS]
        )

    for chunk in X_CHUNKS_SP[:2]:
        load_x(nc.sync, *chunk)

    # ---------------- kernel-generating MLP ----------------
    hid_ps = po2.tile([C, S], f32, tag="oo")
    nc.tensor.matmul(
        hid_ps[:, :], lhsT=wk1t[:, :], rhs=dts[:, :], start=True, stop=True,
    )
    hid_sb = const.tile([C, S], bf16)
    nc.scalar.activation(hid_sb[:, :], hid_ps[:, :], LRELU, alpha=0.1)

    # kern tap columns: kcols[s*64+c, t] = kern[s, c*9+t].
    # All 9 tap matmuls write one psum tile; two strided copies pick the
    # sample-matched column per partition half.
    kps = po2.tile([2 * C, 2 * KK * KK], f32, tag="oo")
    for t in range(KK * KK):
        nc.tensor.matmul(
            kps[:, 2 * t : 2 * t + 2],
            lhsT=wk2td[:, t * 128 : (t + 1) * 128],
            rhs=hid_sb[:, :],
            start=True, stop=True,
        )
    kcols = const.tile([2 * C, KK * KK], f32)
    kps3 = kps[:, :].rearrange("p (t s) -> p t s", s=2)
    nc.vector.tensor_copy(kcols[0:C, :], kps3[0:C, :, 0])
    nc.vector.tensor_copy(kcols[C : 2 * C, :], kps3[C : 2 * C, :, 1])

    for chunk in X_CHUNKS_SP[2:]:
        load_x(nc.sync, *chunk)
    wcb = const.tile([2 * C, 2 * C], bf16)
    nc.sync.dma_start(wcb[:, :], wcb_d)
    bc2 = const.tile([2 * C, 1], f32)
    nc.sync.dma_start(bc2[:, :], bc_d)

    # identity -> per-tap diagonal weight matrices diag[:, t*128:(t+1)*128]
    id_i = const.tile([128, 128], i32)
    nc.gpsimd.iota(id_i[:, :], pattern=[[1, 128]], base=0, channel_multiplier=-1)
    idf = const.tile([128, 128], f32)
    nc.vector.tensor_scalar(idf[:, :], id_i[:, :], 0, None, mybir.AluOpType.is_equal)
    diag = const.tile([128, KK * KK * 128], bf16)
    for t in range(KK * KK):
        nc.vector.tensor_scalar_mul(
            diag[:, t * 128 : (t + 1) * 128], idf[:, :], kcols[:, t : t + 1]
        )

    # ---------------- main loop ----------------
    xrows = xs[:, :].rearrange("p (r w) -> p r w", w=RS)

    def win(r0, nr, di, dj):
        # image rows r0..r0+nr-1 under tap (di,dj); padded row r0+di covers
        # image row r0+di-1 (the +1 pad offset cancels the tap's -1).
        return xrows[:, r0 + di : r0 + di + nr, dj : dj + W]

    dg = {}  # image row -> (tile, px offset) for 4-row (512 px) slices

    def set_dg(r0, nr, tilev, base=0):
        for i in range(nr // 4):
            dg[r0 + 4 * i] = (tilev, base + 512 * i)

    pcur = {"t": None}

    def pe_group(gi, r0):
        # two groups share one [128,1024] psum tile (2 banks)
        if gi % 2 == 0:
            pcur["t"] = pdw.tile([128, 1024], f32, tag="pdw", name=f"pdw{r0}")
        P = pcur["t"]
        half = 512 * (gi % 2)
        for t, (di, dj) in enumerate(TAPS):
            nc.tensor.matmul(
                P[:, half : half + 512],
                lhsT=diag[:, t * 128 : (t + 1) * 128],
                rhs=win(r0, 4, di, dj),
                start=(t == 0), stop=(t == KK * KK - 1),
            )
        if gi % 2 == 1 or gi == len(PE_GROUPS) - 1:
            npx = half + 512
            rbase = r0 - 4 * (gi % 2)
            D = dgp.tile([128, npx], bf16, name=f"dpe{rbase}")
            nc.scalar.activation(D[:, 0:npx], P[:, 0:npx], LRELU, alpha=0.1)
            set_dg(rbase, npx // 128, D)

    def dve_muls(r0, nr, pool):
        px = nr * W
        prod = pool.tile([128, 9 * px], bf16, tag="prod", name=f"prod{r0}")
        p3 = prod[:, :].rearrange("p (t x) -> p t x", x=px)
        for t, (di, dj) in enumerate(TAPS):
            o = p3[:, t, :].rearrange("p (r w) -> p r w", w=W)
            nc.vector.tensor_scalar_mul(o, win(r0, nr, di, dj), kcols[:, t : t + 1])
        return p3

    def adds_and_lrelu(eng, p3, r0, nr, tag, lrelu_dve=False):
        px = nr * W
        for dst, src in ADD_TREE[:-1]:
            eng.tensor_tensor(
                p3[:, dst, :], p3[:, dst, :], p3[:, src, :], op=mybir.AluOpType.add
            )
        acc = accp.tile([128, px], bf16, tag=tag, name=f"acc{r0}")
        eng.tensor_tensor(
            acc[:, :], p3[:, 0, :], p3[:, 8, :], op=mybir.AluOpType.add
        )
        D = dgp.tile([128, px], bf16, name=f"dd{r0}")
        if lrelu_dve:
            # lrelu(v) = max(v, 0.1v) on DVE keeps the chain on one engine
            nc.vector.scalar_tensor_tensor(
                D[:, :], acc[:, :], 0.1, acc[:, :],
                op0=mybir.AluOpType.mult, op1=mybir.AluOpType.max,
            )
        else:
            nc.scalar.activation(D[:, :], acc[:, :], LRELU, alpha=0.1)
        set_dg(r0, nr, D)

    # --- 1x1 span (8 rows = 1024 px) + bias evac; out DMA per span pair ---
    ost_tiles = {}
    ost_done = {}

    def span_1x1(s, evac_dve=False, out_pool=False):
        r0 = 8 * s
        O = po2.tile([128, 1024], f32, tag="oo", name=f"o2{s}")
        for h in range(2):
            t_, off = dg[r0 + 4 * h]
            nc.tensor.matmul(
                O[:, 512 * h : 512 * (h + 1)],
                lhsT=wcb[:, :], rhs=t_[:, off : off + 512],
                start=True, stop=True,
            )
        q = s // 2
        if q not in ost_tiles:
            ost_tiles[q] = ostg.tile([128, 2048], bf16, tag="ostg", name=f"ostg{q}")
            ost_done[q] = 0
        z = ost_tiles[q]
        zsl = z[:, 1024 * (s % 2) : 1024 * (s % 2 + 1)]
        if evac_dve:
            nc.vector.tensor_scalar_add(zsl, O[:, :], bc2[:, 0:1])
        else:
            nc.scalar.add(zsl, O[:, :], bc2[:, 0:1])
        ost_done[q] += 1
        if ost_done[q] == 2:
            eng = nc.gpsimd if out_pool else nc.sync
            eng.dma_start(out_d[:, q * 2048 : (q + 1) * 2048], z[:, :])

    # ---------------- schedule (virtual-time ordered emission) ----------
    # Engines execute their streams near-order with a small lookahead, so
    # emit every op at its estimated ready time to avoid head-of-line
    # convoys.  Costs in us, from the TRN2 cost model.
    MUL_C = lambda px: (px * 0.268 + 105) / 1000.0
    ADD_C = lambda px: (px * 0.53 + 105) / 1000.0
    PADD_C = lambda px: (px * 0.833 + 131) / 1000.0

    events = []  # (vtime, seq, emit_fn)
    seq = [0]

    def ev(vt, fn):
        events.append((vt, seq[0], fn))
        seq[0] += 1

    row_ready = {}  # image row (mult of 4) -> vtime its D tile is ready

    # PE dw groups: start ~5.0, ~1.94us each; prelu lands with the pair.
    vt = 5.0
    for gi, r0 in enumerate(PE_GROUPS):
        vt += 1.94
        ev(vt, (lambda gi=gi, r0=r0: pe_group(gi, r0)))
        if gi % 2 == 1 or gi == len(PE_GROUPS) - 1:
            rbase = r0 - 4 * (gi % 2)
            for rr in range(rbase, r0 + 4, 4):
                row_ready[rr] = vt + 0.9

    # DVE: DP muls first, then D chunks (muls+adds).  DVE clock starts ~5.
    dvt = 5.0
    for r0, nr in DP_CHUNKS:
        dvt += 9 * MUL_C(nr * W)
        ev(dvt - 9 * MUL_C(nr * W),
           (lambda r0=r0, nr=nr: dp_p3.__setitem__(r0, dve_muls(r0, nr, dpprod))))
    dp_mul_done = {}
    dvt2 = 5.0
    for r0, nr in DP_CHUNKS:
        dvt2 += 9 * MUL_C(nr * W)
        dp_mul_done[r0] = dvt2
    for ci, (r0, nr) in enumerate(D_CHUNKS):
        cost = 9 * MUL_C(nr * W) + 8 * ADD_C(nr * W)
        last = ci == len(D_CHUNKS) - 1
        ev(dvt, (lambda r0=r0, nr=nr, last=last: d_chunk(r0, nr, last)))
        dvt += cost
        for rr in range(r0, r0 + nr, 4):
            row_ready[rr] = dvt + 1.2

    # Pool: add trees, serial, gated by the DP muls.
    pvt = 0.0
    for r0, nr in DP_CHUNKS:
        pvt = max(pvt, dp_mul_done[r0])
        ev(pvt, (lambda r0=r0, nr=nr: dp_adds(r0, nr)))
        pvt += 8 * PADD_C(nr * W)
        for rr in range(r0, r0 + nr, 4):
            row_ready[rr] = pvt + 1.2

    # 1x1 spans at max over their two D tiles' readiness.  Late spans use
    # DVE for the bias evac (Act is the convoy then) and the Pool DMA queue
    # for the final output pairs.
    for s in range(16):
        rt = max(row_ready[8 * s], row_ready[8 * s + 4])
        ev(rt, (lambda s=s, rt=rt: span_1x1(s, evac_dve=(rt > 42.0))))

    dp_p3 = {}

    def dp_adds(r0, nr):
        adds_and_lrelu(nc.gpsimd, dp_p3[r0], r0, nr, "pacc")

    def d_chunk(r0, nr, last=False):
        p3 = dve_muls(r0, nr, dprod)
        adds_and_lrelu(nc.vector, p3, r0, nr, "dacc", lrelu_dve=last)

    for _, _, fn in sorted(events, key=lambda e: (e[0], e[1])):
        fn()


# ---------------------------------------------------------------------------
# host-side entry point
# ---------------------------------------------------------------------------

_PROGRAM_CACHE: dict[str, bass.Bass] = {}


def _get_program() -> bass.Bass:
    if "p" not in _PROGRAM_CACHE:
        _PROGRAM_CACHE["p"] = build_program()
    return _PROGRAM_CACHE["p"]


def _host_prep(inputs: dict):
    import ml_dtypes

    x = np.asarray(inputs["x"], dtype=np.float32)
    d = np.asarray(inputs["d"], dtype=np.float32)
    Wk1 = np.asarray(inputs["Wk1"], dtype=np.float32)
    Wk2 = np.asarray(inputs["Wk2"], dtype=np.float32)
    Wc = np.asarray(inputs["Wc"], dtype=np.float32)
    bc = np.asarray(inputs["bc"], dtype=np.float32)

    wk1t = np.ascontiguousarray(Wk1.T).astype(ml_dtypes.bfloat16)
    w = Wk2.reshape(C, KK * KK, C).transpose(2, 1, 0)  # (j, t, c)
    wk2td = np.ascontiguousarray(
        np.concatenate([w, w], axis=2).reshape(C, KK * KK * 2 * C)
    ).astype(ml_dtypes.bfloat16)
    wcb = np.zeros((2 * C, 2 * C), dtype=np.float32)
    wcb[0:C, 0:C] = Wc.T
    wcb[C:, C:] = Wc.T
    wcb = wcb.astype(ml_dtypes.bfloat16)
    bc2 = np.ascontiguousarray(np.concatenate([bc, bc]).reshape(2 * C, 1))

    # host-side zero-padding: [S*C, RP, RS] with image at [1:H+1, 1:W+1]
    B = x.shape[0]
    xpad = np.zeros((B, C, RP, RS), dtype=ml_dtypes.bfloat16)
    xpad[:, :, 1 : H + 1, 1 : W + 1] = x.astype(ml_dtypes.bfloat16)

    in_maps = []
    for i in range(NCORES):
        xp = np.ascontiguousarray(
            xpad[S * i : S * (i + 1)].reshape(S * C, XFREE)
        )
        dT = np.ascontiguousarray(d[S * i : S * (i + 1)].T).astype(ml_dtypes.bfloat16)
        in_maps.append(
            {
                "xpad": xp,
                "dT": dT,
                "wk1t": wk1t,
                "wk2td": wk2td,
                "wcb": wcb,
                "bc2": bc2,
            }
        )
    return in_maps


def run_on_hw(inputs: dict, **kwargs):
    """Run the SPMD kernel on 8 NeuronCores; returns (output, results)."""
    from concourse.bass_utils import run_bass_kernel_spmd

    nc = _get_program()
    in_maps = _host_prep(inputs)
    res = run_bass_kernel_spmd(nc, in_maps, core_ids=list(range(NCORES)), **kwargs)
    outs = res.results
    B = S * NCORES
    out = np.empty((B, C, H, W), dtype=np.float32)
    for i in range(NCORES):
        out[S * i : S * (i + 1)] = outs[i]["out"].astype(np.float32).reshape(
            S, C, H, W
        )
    return out, res


def kernel(**inputs) -> np.ndarray:
    out, _ = run_on_hw(inputs)
    return out


if __name__ == "__main__":
    nc = build_program()
    print("program built OK")
